# revision 1
# baseline (speedup 1.0000x reference)
"""Trainium2 Bass kernel for nn_BottleneckR (pre-activation ternary bottleneck).

Reference (per batch of 64):
  h  = conv1x1(BN1(x), tern(w1))            [64,256,28,28]
  h  = conv3x3s2p1(BN2(h), tern(w2))        [64,256,14,14]
  h  = conv1x1(BN3(h), tern(w3))            [64,1024,14,14]
  sc = BN_ds(conv1x1s2(x, ds_w))            [64,1024,14,14]
  out = h + sc

Strategy: data-parallel over batch on 8 NeuronCores (8 images/core).
Training-mode BN statistics are exact sync-BN: per-channel sum/sumsq are
computed locally and combined with 3 small AllReduces (x+ds stats can be
batched together; BN2/BN3 stats are sequentially dependent).

Math folding (exact, not approximate):
 - tern(w) = alpha_o * S where S in {-1,0,+1}. S is exactly representable
   in any float dtype, so convs run as S-matmuls; alpha folds into the
   *next* BN's affine (BN(a*h + c) == BN(h) up to the eps term, which is
   corrected exactly: a2 = g2*alpha1*rsqrt(alpha1^2*var(p1)+eps)).
 - BN1 folds into conv1 weights per input channel (1x1 conv, no padding,
   so the beta shift is a per-output-channel constant that the next BN
   removes exactly). conv1 therefore reads raw x.
 - conv2/conv3 read explicitly materialized BN outputs (3x3 padding makes
   shift-folding borders wrong, and materializing is one fused DVE op).
 - Matmuls run in float32r (full PE rate). The ternary weight operand is
   exact under f32r truncation; only the moving operand loses ~2^-13.

The ternary sign matrices and alphas depend only on the weights, so they
are folded on the host (standard deployment-style weight preprocessing);
all data-dependent compute (BN stats, affines, convs, residual) is on
device.
"""

import sys

sys.path.insert(0, "/opt/trn_rl_repo")

import numpy as np

import concourse.bacc as bacc
import concourse.mybir as mybir
import concourse.tile as tile
from concourse.bass_utils import run_bass_kernel_spmd

F32 = mybir.dt.float32
F32R = mybir.dt.float32r
BF16 = mybir.dt.bfloat16
AX = mybir.AxisListType
OP = mybir.AluOpType
ACT = mybir.ActivationFunctionType

NCORES = 8
NI = 8            # images per core
C0, C1, C2 = 512, 256, 1024
H1, W1 = 28, 28
H2, W2 = 14, 14
HW1, HW2 = H1 * W1, H2 * W2            # 784, 196
PIX1, PIX2 = NI * HW1, NI * HW2        # 6272, 1568
K0, KM1, KM2 = C0 // 128, C1 // 128, C2 // 128   # 4, 2, 8
NCH = 392                               # pixel chunk (one image pair at 14x14)
NG1 = 64 * HW1                          # global BN count at 28x28
NG2 = 64 * HW2                          # global BN count at 14x14
EPS = 1e-5

# phase-split xn2 layout (for conv2's stride-2 taps):
# p1 tile = [128, P1PAD + 4 planes x (8 imgs x 15 rows x 14 cols)]
# plane (by,bx) holds xn2 phase iy=2*ay+by, ix=2*ax+bx; row ay=14 is a zero
# guard so dy=0 taps read [offset -14] and get zeros for oy=0.
P1_IMG = 15 * 14          # 210 elems per image incl guard row
P1_PLANE = NI * P1_IMG    # 1680
P1PAD = 14                # leading zero pad (first y-offset read)
P1TOT = P1PAD + 4 * P1_PLANE   # 6734

_CACHE = {}


def _ternarize_host(w):
    """fp32 ternarize matching the jax reference: returns (S, alpha)."""
    w = np.asarray(w, np.float32)
    absw = np.abs(w)
    delta = (0.7 * absw.mean(axis=(1, 2, 3), keepdims=True)).astype(np.float32)
    mask = (absw > delta).astype(np.float32)
    alpha = (absw * mask).sum(axis=(1, 2, 3)) / (mask.sum(axis=(1, 2, 3)) + 1e-8)
    sign = np.sign(w).astype(np.float32)
    return (sign * mask).astype(np.float32), alpha.astype(np.float32)


def _r(ap):
    return ap.bitcast(F32R)


def build_program():
    nc = bacc.Bacc("TRN2", target_bir_lowering=False, debug=False,
                   num_devices=NCORES)

    # ---- DRAM I/O (per core) ----
    x_d = nc.dram_tensor("x", [C0, PIX1], F32R, kind="ExternalInput")
    t1_d = nc.dram_tensor("t1", [C0, C1], F32R, kind="ExternalInput")
    t2_d = nc.dram_tensor("t2", [9, C1, C1], F32R, kind="ExternalInput")
    t3_d = nc.dram_tensor("t3", [C1, C2], F32R, kind="ExternalInput")
    dsw_d = nc.dram_tensor("dsw", [C0, C2], F32R, kind="ExternalInput")
    a1_d = nc.dram_tensor("alpha1", [C1], F32, kind="ExternalInput")
    a2_d = nc.dram_tensor("alpha2", [C1], F32, kind="ExternalInput")
    a3_d = nc.dram_tensor("alpha3", [C2], F32, kind="ExternalInput")
    bn1g_d = nc.dram_tensor("bn1g", [C0], F32, kind="ExternalInput")
    bn2g_d = nc.dram_tensor("bn2g", [C1], F32, kind="ExternalInput")
    bn2b_d = nc.dram_tensor("bn2b", [C1], F32, kind="ExternalInput")
    bn3g_d = nc.dram_tensor("bn3g", [C1], F32, kind="ExternalInput")
    bn3b_d = nc.dram_tensor("bn3b", [C1], F32, kind="ExternalInput")
    dsg_d = nc.dram_tensor("dsg", [C2], F32, kind="ExternalInput")
    dsb_d = nc.dram_tensor("dsb", [C2], F32, kind="ExternalInput")
    out_d = nc.dram_tensor("out", [C2, PIX2], F32, kind="ExternalOutput")

    def colview(dram, m):
        # [m*128] dram vector -> SBUF [128, m] column tile access pattern
        return dram.ap().rearrange("(m p) -> p m", p=128)

    with tile.TileContext(nc) as tc:
        _build_tile_program(
            nc, tc,
            x_d, t1_d, t2_d, t3_d, dsw_d,
            a1_d, a2_d, a3_d,
            bn1g_d, bn2g_d, bn2b_d, bn3g_d, bn3b_d, dsg_d, dsb_d,
            out_d, colview,
        )

    nc.compile()
    return nc



def _build_tile_program(nc, tc, x_d, t1_d, t2_d, t3_d, dsw_d,
                        a1_d, a2_d, a3_d,
                        bn1g_d, bn2g_d, bn2b_d, bn3g_d, bn3b_d, dsg_d, dsb_d,
                        out_d, colview):
    from contextlib import ExitStack

    gctx = ExitStack()
    with gctx:
        dram = gctx.enter_context(tc.tile_pool(name="dram", bufs=1, space="DRAM"))
        sb_w = gctx.enter_context(tc.tile_pool(name="sb_w", bufs=1))
        sb_big = gctx.enter_context(tc.tile_pool(name="sb_big", bufs=1))
        sb_small = gctx.enter_context(tc.tile_pool(name="sb_small", bufs=1))
        ps_mm = gctx.enter_context(tc.tile_pool(name="ps_mm", bufs=4, space="PSUM"))
        ps_c2 = gctx.enter_context(tc.tile_pool(name="ps_c2", bufs=4, space="PSUM"))

        # ------------- static loads -------------
        t1 = []
        for k in range(K0):
            tk = sb_w.tile([128, C1], F32R, name=f"t1_{k}")
            nc.sync.dma_start(tk[:], t1_d.ap()[k * 128:(k + 1) * 128, :])
            t1.append(tk)

        # per-channel parameter columns
        a1c = sb_small.tile([128, KM1], F32, name="a1c")
        nc.sync.dma_start(a1c[:], colview(a1_d, KM1))
        a2c = sb_small.tile([128, KM1], F32, name="a2c")
        nc.sync.dma_start(a2c[:], colview(a2_d, KM1))
        a3c = sb_small.tile([128, KM2], F32, name="a3c")
        nc.sync.dma_start(a3c[:], colview(a3_d, KM2))
        bn1g = sb_small.tile([128, K0], F32, name="bn1g")
        nc.sync.dma_start(bn1g[:], colview(bn1g_d, K0))
        bn2g = sb_small.tile([128, KM1], F32, name="bn2g")
        nc.sync.dma_start(bn2g[:], colview(bn2g_d, KM1))
        bn2b = sb_small.tile([128, KM1], F32, name="bn2b")
        nc.sync.dma_start(bn2b[:], colview(bn2b_d, KM1))
        bn3g = sb_small.tile([128, KM1], F32, name="bn3g")
        nc.sync.dma_start(bn3g[:], colview(bn3g_d, KM1))
        bn3b = sb_small.tile([128, KM1], F32, name="bn3b")
        nc.sync.dma_start(bn3b[:], colview(bn3b_d, KM1))
        dsg = sb_small.tile([128, KM2], F32, name="dsg")
        nc.sync.dma_start(dsg[:], colview(dsg_d, KM2))
        dsb = sb_small.tile([128, KM2], F32, name="dsb")
        nc.sync.dma_start(dsb[:], colview(dsb_d, KM2))

        # stat tiles
        xbn = sb_small.tile([128, K0 * 16 * 6], F32, name="xbn")
        xagg = sb_small.tile([128, K0 * 2], F32, name="xagg")
        p1bn = sb_small.tile([128, KM1 * 32 * 6], F32, name="p1bn")
        p1agg = sb_small.tile([128, KM1 * 2], F32, name="p1agg")
        qsum_c = sb_small.tile([128, KM2 * 4], F32, name="qsum_c")
        st1 = sb_small.tile([128, 8], F32, name="st1")     # x: sum4, sq4
        g1 = sb_small.tile([128, 8], F32, name="g1")
        p2bn = sb_small.tile([128, KM1 * 4 * 6], F32, name="p2bn")
        p2agg = sb_small.tile([128, KM1 * 2], F32, name="p2agg")
        # st2: p1 sum2, p1 sq2, q sum8, q sq8
        st2 = sb_small.tile([128, 20], F32, name="st2")
        g2 = sb_small.tile([128, 20], F32, name="g2")
        p2s_c = sb_small.tile([128, KM1 * 4], F32, name="p2s_c")
        st3 = sb_small.tile([128, 4], F32, name="st3")
        g3 = sb_small.tile([128, 4], F32, name="g3")

        # persistent activations
        q = [sb_big.tile([128, PIX2], F32, name=f"q_{m}") for m in range(KM2)]
        p1 = [sb_big.tile([128, P1TOT], F32R, name=f"p1_{m}") for m in range(KM1)]

        def pair_view(tile_ap):
            # [128, 1568] (2 imgs) -> [p, i, by, bx, ay, ax]
            return tile_ap.rearrange(
                "p (i ay by ax bx) -> p i by bx ay ax",
                i=2, ay=14, by=2, ax=14, bx=2)

        def load_pair(p, tag_extra=""):
            xp = []
            for k in range(K0):
                tk = sb_stream.tile([128, 2 * HW1], F32R, tag=f"xs{k}",
                                    name=f"xst{tag_extra}_{p}_{k}")
                nc.sync.dma_start(
                    tk[:],
                    x_d.ap()[k * 128:(k + 1) * 128,
                             p * 2 * HW1:(p + 1) * 2 * HW1])
                xp.append(tk)
            return xp

        with tc.tile_pool(name="sb_stream", bufs=2) as sb_stream:
            # ================= phase A: x stats + ds conv =================
            # pairs 0,1 stream through a 1-buf rotation; pairs 2,3 land in
            # resident tiles (kept for conv1, halving the re-stream).
            with tc.tile_pool(name="sb_pA", bufs=1) as sb_pA, \
                 tc.tile_pool(name="sb_dumA", bufs=2) as sb_dumA:
                dsw = []
                for k in range(K0):
                    dk = sb_pA.tile([128, C2], F32R, name=f"dsw_{k}")
                    nc.sync.dma_start(dk[:], dsw_d.ap()[k * 128:(k + 1) * 128, :])
                    dsw.append(dk)

                def phaseA_pair(p, xk_aps):
                    # x bn_stats on DVE
                    for k in range(K0):
                        for g in range(4):
                            slot = k * 16 + p * 4 + g
                            nc.vector.bn_stats(
                                xbn[:, slot * 6:(slot + 1) * 6],
                                xk_aps[k][:, g * 392:(g + 1) * 392].bitcast(F32))
                    # ds conv; psum -> q copy on ACT with sum accumulation
                    for m in range(KM2):
                        pmm = ps_mm.tile([128, NCH], F32, tag="mm",
                                         name=f"psds_{p}_{m}")
                        for k in range(K0):
                            rhs = pair_view(xk_aps[k])[:, :, 0, 0, :, :]
                            nc.tensor.matmul(
                                pmm[:], dsw[k][:, m * 128:(m + 1) * 128],
                                rhs, start=(k == 0), stop=(k == K0 - 1))
                        if m % 2 == 0:
                            nc.vector.tensor_scalar(
                                q[m][:, p * NCH:(p + 1) * NCH], pmm[:],
                                1.0, 0.0, OP.mult, OP.add,
                                accum_out=qsum_c[:, m * 4 + p:m * 4 + p + 1])
                        else:
                            nc.scalar.activation(
                                q[m][:, p * NCH:(p + 1) * NCH], pmm[:],
                                ACT.Copy,
                                accum_out=qsum_c[:, m * 4 + p:m * 4 + p + 1])

                for p in range(4):
                    xp = []
                    for k in range(K0):
                        tk = sb_stream.tile([128, 2 * HW1], F32R, tag=f"xs{k}",
                                            name=f"xstA_{p}_{k}")
                        nc.sync.dma_start(
                            tk[:],
                            x_d.ap()[k * 128:(k + 1) * 128,
                                     p * 2 * HW1:(p + 1) * 2 * HW1])
                        xp.append(tk)
                    phaseA_pair(p, [t[:] for t in xp])

                # x stat aggregation + AR1 (x only)
                xbnv = xbn[:].rearrange("p (kg s) -> p kg s", s=6)
                for k in range(K0):
                    groups = xbnv[:, k * 16:(k + 1) * 16, :]
                    nc.vector.bn_aggr(xagg[:, k * 2:k * 2 + 2], groups)
                xaggv = xagg[:].rearrange("p (k two) -> p k two", two=2)
                mcols = xaggv[:, :, 0]
                vcols = xaggv[:, :, 1]
                msq = sb_small.tile([128, K0], F32, name="msq")
                nc.vector.tensor_tensor(msq[:], mcols, mcols, OP.mult)
                nc.vector.tensor_scalar(st1[:, 0:K0], mcols, float(PIX1), None,
                                        OP.mult)
                nc.vector.tensor_tensor(msq[:], msq[:], vcols, OP.add)
                nc.vector.tensor_scalar(st1[:, K0:2 * K0], msq[:], float(PIX1),
                                        None, OP.mult)
                ar1_in = dram.tile([128, 8], F32, name="ar1_in")
                ar1_out = dram.tile([128, 8], F32, name="ar1_out")
                nc.sync.dma_start(ar1_in[:], st1[:])
                nc.gpsimd.collective_compute(
                    "AllReduce", OP.add, replica_groups=[list(range(NCORES))],
                    ins=[ar1_in.opt()], outs=[ar1_out.opt()])
                nc.sync.dma_start(g1[:], ar1_out[:])

                # q sumsq on ACT (square into dummy, accumulate) + sum reduce
                for m in range(KM2):
                    dums = sb_dumA.tile([128, PIX2], BF16, tag="duma",
                                        name=f"dumq_{m}")
                    nc.scalar.activation(dums[:], q[m][:], ACT.Square,
                                         accum_out=st2[:, 12 + m:12 + m + 1])
                    nc.vector.reduce_sum(st2[:, 4 + m:4 + m + 1],
                                         qsum_c[:, m * 4:(m + 1) * 4], axis=AX.X)

            # ---- BN1 affine -> fold into T1; ds affine ----
            mean_x = sb_small.tile([128, K0], F32, name="mean_x")
            var_x = sb_small.tile([128, K0], F32, name="var_x")
            a1f = sb_small.tile([128, K0], F32, name="a1f")
            tmp_k0 = sb_small.tile([128, K0], F32, name="tmp_k0")
            nc.vector.tensor_scalar(mean_x[:], g1[:, 0:K0], 1.0 / NG1, None, OP.mult)
            nc.vector.tensor_tensor(tmp_k0[:], mean_x[:], mean_x[:], OP.mult)
            nc.vector.tensor_scalar(var_x[:], g1[:, K0:2 * K0], 1.0 / NG1, None,
                                    OP.mult)
            nc.vector.tensor_tensor(var_x[:], var_x[:], tmp_k0[:], OP.subtract)

            def rsqrt_cols(dst, var_ap, gamma_ap, extra_mul=None):
                cols = dst.shape[1]
                tmp = sb_small.tile([128, cols], F32, tag="rsq_tmp",
                                    name=f"rsq_{dst.tensor.name}")
                nc.vector.tensor_scalar(tmp[:], var_ap, EPS, None, OP.add)
                nc.vector.reciprocal(tmp[:], tmp[:])
                nc.scalar.sqrt(tmp[:], tmp[:])
                nc.vector.tensor_tensor(dst, tmp[:], gamma_ap, OP.mult)
                if extra_mul is not None:
                    nc.vector.tensor_tensor(dst, dst, extra_mul, OP.mult)

            rsqrt_cols(a1f[:], var_x[:], bn1g[:])
            for k in range(K0):
                nc.vector.tensor_scalar(t1[k][:], t1[k][:], a1f[:, k:k + 1], None,
                                        OP.mult)

            # ================= conv1 =================
            # resident pairs (imgs 4..7) run right after AR1; pairs 0,1 re-stream.
            zc32 = sb_small.tile([128, 1], F32, name="zc32")
            nc.vector.memset(zc32[:], 0.0)
            zcr = sb_small.tile([128, 1], F32R, name="zcr")
            nc.vector.tensor_copy(zcr[:], zc32[:])

            def zero_f32r(dst_ap):
                shape = [dst_ap.shape[0]] + list(dst_ap.shape[1:])
                nc.vector.tensor_copy(dst_ap, zcr[:].broadcast_to(shape))

            for m in range(KM1):
                zero_f32r(p1[m][:, 0:P1PAD])
                gv = p1[m][:, P1PAD + 196:P1PAD + 196 + 31 * P1_IMG].rearrange(
                    "p (r e) -> p r e", e=P1_IMG)[:, :, 0:14]
                zero_f32r(gv)
                zero_f32r(p1[m][:, P1TOT - 14:P1TOT])

            if True:
                def conv1_pair(p, xk_aps):
                    # xk_aps: [K0] APs of [128, 1568] (one image pair).
                    # Sweep the 4 pixel chunks per (m,k) weight so each
                    # t1 slice is loaded once per pair.
                    pmms = {}
                    for m in range(KM1):
                        for c in range(4):
                            pmms[(m, c)] = ps_mm.tile(
                                [128, NCH], F32, tag="mm",
                                name=f"psc1_{p}_{m}_{c}")
                    for m in range(KM1):
                        for k in range(K0):
                            for c in range(4):
                                nc.tensor.matmul(
                                    pmms[(m, c)][:],
                                    t1[k][:, m * 128:(m + 1) * 128],
                                    xk_aps[k][:, c * NCH:(c + 1) * NCH],
                                    start=(k == 0), stop=(k == K0 - 1))
                    for m in range(KM1):
                        for c in range(4):
                            img = p * 2 + c // 2
                            half = c % 2
                            pmm = pmms[(m, c)]
                            src = pmm[:].rearrange(
                                "p (ay by ax bx) -> p by bx ay ax",
                                ay=7, by=2, ax=14, bx=2)
                            pv = p1[m][:, P1PAD:].rearrange(
                                "p (by bx i ay ax) -> p by bx i ay ax",
                                by=2, bx=2, i=NI, ay=15, ax=14)
                            for by in range(2):
                                dst = pv[:, by, :, img,
                                         half * 7:half * 7 + 7, :]
                                nc.vector.tensor_scalar(
                                    dst, src[:, by], 1.0, None, OP.mult)
                    for m in range(KM1):
                        for img in (p * 2, p * 2 + 1):
                            for pl in range(4):
                                slot = m * 32 + img * 4 + pl
                                blk = p1[m][:,
                                            P1PAD + pl * P1_PLANE + img * P1_IMG:
                                            P1PAD + pl * P1_PLANE
                                            + (img + 1) * P1_IMG]
                                nc.vector.bn_stats(
                                    p1bn[:, slot * 6:(slot + 1) * 6],
                                    blk.bitcast(F32))

                for p in range(4):
                    xp = []
                    for k in range(K0):
                        tk = sb_stream.tile([128, 2 * HW1], F32R, tag=f"xs{k}",
                                            name=f"xstB_{p}_{k}")
                        nc.sync.dma_start(
                            tk[:],
                            x_d.ap()[k * 128:(k + 1) * 128,
                                     p * 2 * HW1:(p + 1) * 2 * HW1])
                        xp.append(tk)
                    conv1_pair(p, [tk[:] for tk in xp])

                # p1 stat aggregation
                for m in range(KM1):
                    groups = p1bn[:, m * 192:(m + 1) * 192].rearrange(
                        "p (g s) -> p g s", s=6)
                    nc.vector.bn_aggr(p1agg[:, m * 2:m * 2 + 2], groups)
                pav = p1agg[:].rearrange("p (m two) -> p m two", two=2)
                pm = pav[:, :, 0]
                pv_ = pav[:, :, 1]
                pmsq = sb_small.tile([128, KM1], F32, name="pmsq")
                NP1 = float(4 * P1_PLANE)
                nc.vector.tensor_tensor(pmsq[:], pm, pm, OP.mult)
                nc.vector.tensor_scalar(st2[:, 0:KM1], pm, NP1, None, OP.mult)
                nc.vector.tensor_tensor(pmsq[:], pmsq[:], pv_, OP.add)
                nc.vector.tensor_scalar(st2[:, KM1:2 * KM1], pmsq[:], NP1,
                                        None, OP.mult)

                # ---- AR2: p1 stats + q stats ----
                ar2_in = dram.tile([128, 20], F32, name="ar2_in")
                ar2_out = dram.tile([128, 20], F32, name="ar2_out")
                nc.sync.dma_start(ar2_in[:], st2[:])
                nc.gpsimd.collective_compute(
                    "AllReduce", OP.add, replica_groups=[list(range(NCORES))],
                    ins=[ar2_in.opt()], outs=[ar2_out.opt()])
                nc.sync.dma_start(g2[:], ar2_out[:])

        # ---- BN2 affine (alpha1-corrected) + xn2 in place ----
        sb_p2 = gctx.enter_context(tc.tile_pool(name="sb_p2", bufs=1))
        p2 = [sb_p2.tile([128, PIX2], F32R, name=f"p2_{m}")
              for m in range(KM1)]
        mean_p1 = sb_small.tile([128, KM1], F32, name="mean_p1")
        var_p1 = sb_small.tile([128, KM1], F32, name="var_p1")
        a2f = sb_small.tile([128, KM1], F32, name="a2f")
        b2f = sb_small.tile([128, KM1], F32, name="b2f")
        tmp_m1 = sb_small.tile([128, KM1], F32, name="tmp_m1")
        nc.vector.tensor_scalar(mean_p1[:], g2[:, 0:2], 1.0 / NG1, None, OP.mult)
        nc.vector.tensor_scalar(var_p1[:], g2[:, 2:4], 1.0 / NG1, None, OP.mult)

        # ds BN affine (stats arrived with AR2; needed only at the fuse)
        mean_q = sb_small.tile([128, KM2], F32, name="mean_q")
        var_q = sb_small.tile([128, KM2], F32, name="var_q")
        aq = sb_small.tile([128, KM2], F32, name="aq")
        bq = sb_small.tile([128, KM2], F32, name="bq")
        tmp_m2 = sb_small.tile([128, KM2], F32, name="tmp_m2")
        nc.vector.tensor_scalar(mean_q[:], g2[:, 4:12], 1.0 / NG2, None, OP.mult)
        nc.vector.tensor_scalar(var_q[:], g2[:, 12:20], 1.0 / NG2, None, OP.mult)
        nc.vector.tensor_tensor(tmp_m2[:], mean_q[:], mean_q[:], OP.mult)
        nc.vector.tensor_tensor(var_q[:], var_q[:], tmp_m2[:], OP.subtract)
        rsqrt_cols(aq[:], var_q[:], dsg[:])
        nc.vector.tensor_tensor(tmp_m2[:], aq[:], mean_q[:], OP.mult)
        nc.vector.tensor_tensor(bq[:], dsb[:], tmp_m2[:], OP.subtract)
        nc.vector.tensor_tensor(tmp_m1[:], mean_p1[:], mean_p1[:], OP.mult)
        nc.vector.tensor_tensor(var_p1[:], var_p1[:], tmp_m1[:], OP.subtract)
        nc.vector.tensor_tensor(tmp_m1[:], a1c[:], a1c[:], OP.mult)
        nc.vector.tensor_tensor(var_p1[:], var_p1[:], tmp_m1[:], OP.mult)
        rsqrt_cols(a2f[:], var_p1[:], bn2g[:], extra_mul=a1c[:])
        nc.vector.tensor_tensor(tmp_m1[:], a2f[:], mean_p1[:], OP.mult)
        nc.vector.tensor_tensor(b2f[:], bn2b[:], tmp_m1[:], OP.subtract)
        for m in range(KM1):
            dv = p1[m][:, P1PAD:].rearrange(
                "p (pl i ay ax) -> p (pl i) ay ax", pl=4, i=NI, ay=15, ax=14)
            dv = dv[:, :, 0:14, :]     # data rows only; guards stay zero
            nc.vector.tensor_scalar(dv, dv, a2f[:, m:m + 1],
                                    b2f[:, m:m + 1], OP.mult, OP.add)

        # ================= conv2: 3x3 s2 p1 =================
        # tap -> (plane source, y-offset). planes: 0=P00 1=P01 2=P10 3=P11,
        # 4=C=xshift(P01), 5=D=xshift(P11). y-offset -14 reads guard zeros
        # for oy=0 (dy=0 taps).
        TAPS = [
            (5, -14), (2, -14), (3, -14),     # dy=0: dx=0,1,2
            (4, 0),   (0, 0),   (1, 0),       # dy=1
            (5, 0),   (2, 0),   (3, 0),       # dy=2
        ]
        with tc.tile_pool(name="sb_pC", bufs=1) as sb_pC, \
             tc.tile_pool(name="sb_cd", bufs=1) as sb_cd:
            t2 = []
            for k in range(KM1):
                tk = sb_pC.tile([128, 9, C1], F32R, name=f"t2_{k}")
                src = t2_d.ap()[:, k * 128:(k + 1) * 128, :].rearrange(
                    "t p o -> p t o")
                nc.sync.dma_start(tk[:], src)
                t2.append(tk)

            def plane_pair_ap(k, plane, p, yoff, cd=None):
                # [128, 2, 196] view of (plane, image-pair p) with y offset
                if cd is None:
                    base = P1PAD + plane * P1_PLANE + p * 2 * P1_IMG + yoff
                    v = p1[k][:, base:base + 2 * P1_IMG]
                else:
                    base = P1PAD + p * 2 * P1_IMG + yoff
                    v = cd[:, base:base + 2 * P1_IMG]
                return v.rearrange("p (i e) -> p i e", i=2, e=P1_IMG)[:, :, 0:HW2]

            # full-size x-shifted planes C=xshift(P01), D=xshift(P11)
            cd_full = {}
            for k in range(KM1):
                for src_plane, nm in ((1, "C"), (3, "D")):
                    cd = sb_cd.tile([128, P1PAD + P1_PLANE], F32R,
                                    tag=f"cd{nm}{k}", name=f"cd{nm}_{k}")
                    zero_f32r(cd[:])
                    sbase = P1PAD + src_plane * P1_PLANE
                    sv = p1[k][:, sbase:sbase + P1_PLANE].rearrange(
                        "p (i ay ax) -> p i ay ax", i=NI, ay=15, ax=14)
                    dvv = cd[:, P1PAD:].rearrange(
                        "p (i ay ax) -> p i ay ax", i=NI, ay=15, ax=14)
                    nc.vector.tensor_scalar(
                        dvv[:, :, 0:14, 1:14], sv[:, :, 0:14, 0:13],
                        1.0, None, OP.mult)
                    cd_full[(4 + (0 if nm == "C" else 1), k)] = cd

            # inverted loops: each t2 weight slice loads once, serves all
            # 4 image pairs (psum per (ip) rotates through 4 banks)
            for m in range(KM1):
                pmms = [ps_c2.tile([128, NCH], F32, tag="c2",
                                   name=f"psc2_{m}_{ip}") for ip in range(4)]
                for t_idx, (plane, yoff) in enumerate(TAPS):
                    for k in range(KM1):
                        for ip in range(4):
                            if plane >= 4:
                                rhs = plane_pair_ap(
                                    k, plane, ip, yoff,
                                    cd=cd_full[(plane, k)][:])
                            else:
                                rhs = plane_pair_ap(k, plane, ip, yoff)
                            last = (t_idx == 8 and k == KM1 - 1)
                            nc.tensor.matmul(
                                pmms[ip][:],
                                t2[k][:, t_idx, m * 128:(m + 1) * 128],
                                rhs, start=(t_idx == 0 and k == 0), stop=last)
                for ip in range(4):
                    nc.vector.tensor_scalar(
                        p2[m][:, ip * NCH:(ip + 1) * NCH], pmms[ip][:],
                        1.0, 0.0, OP.mult, OP.add,
                        accum_out=p2s_c[:, m * 4 + ip:m * 4 + ip + 1])

            # p2 stats
            for m in range(KM1):
                nc.vector.reduce_sum(st3[:, m:m + 1],
                                     p2s_c[:, m * 4:(m + 1) * 4], axis=AX.X)
                for g in range(4):
                    slot = m * 4 + g
                    nc.vector.bn_stats(
                        p2bn[:, slot * 6:(slot + 1) * 6],
                        p2[m][:, g * 392:(g + 1) * 392].bitcast(F32))
                nc.vector.bn_aggr(
                    p2agg[:, m * 2:m * 2 + 2],
                    p2bn[:, m * 24:(m + 1) * 24].rearrange(
                        "p (g s) -> p g s", s=6))
            p2av = p2agg[:].rearrange("p (m two) -> p m two", two=2)
            p2msq = sb_small.tile([128, KM1], F32, name="p2msq")
            nc.vector.tensor_tensor(p2msq[:], p2av[:, :, 0], p2av[:, :, 0],
                                    OP.mult)
            nc.vector.tensor_tensor(p2msq[:], p2msq[:], p2av[:, :, 1], OP.add)
            nc.vector.tensor_scalar(st3[:, 2:4], p2msq[:], float(PIX2),
                                    None, OP.mult)

            # ---- AR3 ----
            ar3_in = dram.tile([128, 4], F32, name="ar3_in")
            ar3_out = dram.tile([128, 4], F32, name="ar3_out")
            nc.sync.dma_start(ar3_in[:], st3[:])
            nc.gpsimd.collective_compute(
                "AllReduce", OP.add, replica_groups=[list(range(NCORES))],
                ins=[ar3_in.opt()], outs=[ar3_out.opt()])
            nc.sync.dma_start(g3[:], ar3_out[:])

        # shortcut affine in place on q (fills the AR3 latency window;
        # depends only on AR2 stats)
        for m in range(KM2):
            nc.vector.tensor_scalar(q[m][:], q[m][:], aq[:, m:m + 1],
                                    bq[:, m:m + 1], OP.mult, OP.add)

        # ---- BN3 affine (alpha2-corrected) + xn3 in place ----
        mean_p2 = sb_small.tile([128, KM1], F32, name="mean_p2")
        var_p2 = sb_small.tile([128, KM1], F32, name="var_p2")
        a3f = sb_small.tile([128, KM1], F32, name="a3f")
        b3f = sb_small.tile([128, KM1], F32, name="b3f")
        tmp_m3 = sb_small.tile([128, KM1], F32, name="tmp_m3")
        nc.vector.tensor_scalar(mean_p2[:], g3[:, 0:2], 1.0 / NG2, None, OP.mult)
        nc.vector.tensor_scalar(var_p2[:], g3[:, 2:4], 1.0 / NG2, None, OP.mult)
        nc.vector.tensor_tensor(tmp_m3[:], mean_p2[:], mean_p2[:], OP.mult)
        nc.vector.tensor_tensor(var_p2[:], var_p2[:], tmp_m3[:], OP.subtract)
        nc.vector.tensor_tensor(tmp_m3[:], a2c[:], a2c[:], OP.mult)
        nc.vector.tensor_tensor(var_p2[:], var_p2[:], tmp_m3[:], OP.mult)
        rsqrt_cols(a3f[:], var_p2[:], bn3g[:], extra_mul=a2c[:])
        nc.vector.tensor_tensor(tmp_m3[:], a3f[:], mean_p2[:], OP.mult)
        nc.vector.tensor_tensor(b3f[:], bn3b[:], tmp_m3[:], OP.subtract)
        for m in range(KM1):
            nc.vector.tensor_scalar(p2[m][:], p2[m][:], a3f[:, m:m + 1],
                                    b3f[:, m:m + 1], OP.mult, OP.add)

        # ================= conv3 + residual fuse + store =================
        with tc.tile_pool(name="sb_pD", bufs=1) as sb_pD, \
             tc.tile_pool(name="sb_stage", bufs=4) as sb_stage:
            t3 = []
            for k in range(KM1):
                tk = sb_pD.tile([128, C2], F32R, name=f"t3_{k}")
                nc.sync.dma_start(tk[:], t3_d.ap()[k * 128:(k + 1) * 128, :])
                t3.append(tk)

            for m in range(KM2):
                for j in range(4):
                    pmm = ps_mm.tile([128, NCH], F32, tag="mm",
                                     name=f"psc3_{m}_{j}")
                    for k in range(KM1):
                        nc.tensor.matmul(
                            pmm[:], t3[k][:, m * 128:(m + 1) * 128],
                            p2[k][:, j * NCH:(j + 1) * NCH],
                            start=(k == 0), stop=(k == KM1 - 1))
                    out_t = sb_stage.tile([128, NCH], F32, tag="out",
                                          name=f"out_{m}_{j}")
                    nc.vector.scalar_tensor_tensor(
                        out_t[:], pmm[:], a3c[:, m:m + 1],
                        q[m][:, j * NCH:(j + 1) * NCH], OP.mult, OP.add)
                    nc.sync.dma_start(
                        out_d.ap()[m * 128:(m + 1) * 128,
                                   j * NCH:(j + 1) * NCH],
                        out_t[:])


def _prep_host(inputs):
    """Host-side: shard x, fold weights, build per-core in_maps."""
    x = np.asarray(inputs["x"], np.float32)          # [64, 512, 28, 28]
    w1 = np.asarray(inputs["w1"], np.float32)
    w2 = np.asarray(inputs["w2"], np.float32)
    w3 = np.asarray(inputs["w3"], np.float32)
    ds_w = np.asarray(inputs["ds_w"], np.float32)

    s1, al1 = _ternarize_host(w1)    # [256,512,1,1]
    s2, al2 = _ternarize_host(w2)    # [256,256,3,3]
    s3, al3 = _ternarize_host(w3)    # [1024,256,1,1]

    t1 = np.ascontiguousarray(s1[:, :, 0, 0].T)                  # [512, 256]
    # t2[t, i, o] = s2[o, i, ky, kx], t = ky*3+kx
    t2 = np.ascontiguousarray(s2.transpose(2, 3, 1, 0).reshape(9, C1, C1))
    t3 = np.ascontiguousarray(s3[:, :, 0, 0].T)                  # [256, 1024]
    dsw = np.ascontiguousarray(ds_w[:, :, 0, 0].T)               # [512, 1024]

    common = dict(
        t1=t1, t2=t2, t3=t3, dsw=dsw,
        alpha1=al1, alpha2=al2, alpha3=al3,
        bn1g=np.asarray(inputs["bn1_g"], np.float32),
        bn2g=np.asarray(inputs["bn2_g"], np.float32),
        bn2b=np.asarray(inputs["bn2_b"], np.float32),
        bn3g=np.asarray(inputs["bn3_g"], np.float32),
        bn3b=np.asarray(inputs["bn3_b"], np.float32),
        dsg=np.asarray(inputs["ds_bn_g"], np.float32),
        dsb=np.asarray(inputs["ds_bn_b"], np.float32),
    )

    in_maps = []
    for c in range(NCORES):
        xs = x[c * NI:(c + 1) * NI]                      # [8, 512, 28, 28]
        xs = np.ascontiguousarray(
            xs.transpose(1, 0, 2, 3).reshape(C0, PIX1))  # [512, 6272]
        in_maps.append({"x": xs, **common})
    return in_maps


def kernel(**inputs):
    if "nc" not in _CACHE:
        _CACHE["nc"] = build_program()
    nc = _CACHE["nc"]

    in_maps = _prep_host(inputs)
    try:
        res = run_bass_kernel_spmd(nc, in_maps, core_ids=list(range(NCORES)))
    except Exception:
        # transient device state (e.g. a previous crashed run) usually
        # clears on retry
        res = run_bass_kernel_spmd(nc, in_maps, core_ids=list(range(NCORES)))

    out = np.empty((64, C2, H2, W2), np.float32)
    for c in range(NCORES):
        oc = res.results[c]["out"].reshape(C2, NI, H2, W2)
        out[c * NI:(c + 1) * NI] = oc.transpose(1, 0, 2, 3)
    return out



# revision 8
# speedup vs baseline: 1.1620x; 1.1620x over previous
"""Trainium2 Bass kernel for nn_BottleneckR (pre-activation ternary bottleneck).

Reference (batch 64):
  h  = conv1x1(BN1(x), tern(w1))            [64,256,28,28]
  h  = conv3x3s2p1(BN2(h), tern(w2))        [64,256,14,14]
  h  = conv1x1(BN3(h), tern(w3))            [64,1024,14,14]
  sc = BN_ds(conv1x1s2(x, ds_w))            [64,1024,14,14]
  out = h + sc

Strategy: data-parallel over batch on 8 NeuronCores (8 images/core).
Exact sync-BN via 3 small AllReduces of per-channel sum/sumsq.

Differences vs the v1 kernel (306us):
 - bf16 end to end: x is converted to bf16 on the host (halves the HBM
   stream), all matmuls run bf16 at full PE rate, DVE elementwise ops get
   the 2x 16-bit mode. Ternary sign matrices are exact in bf16.
 - x stays resident in SBUF (one HBM read instead of two).
 - p1 is stored in a 29x29 zero-padded per-image layout; conv2's 9 taps
   read it directly with stride-2 access patterns (no phase-split scatter,
   no shifted-plane copies).
 - BN2 is folded into conv2 instead of materialized: the scale a2f folds
   into the t2 weights (per input channel), the shift b2f is handled by
   adding T[o] = sum_i S2sum[i,o]*b2f[i] at the psum->p2 copy while the
   pad cells hold -b2f/a2f (so border taps contribute zero shift, exactly
   matching zero padding after BN).
 - AR1 is triggered as soon as the x statistics are done; the ds-conv
   tail, q sum-of-squares and all q/p1 copies overlap the collective.
 - q's BN affine runs during AR3's latency window.
 - One persistent tile pool: no mid-kernel pool-exit barriers (the v1
   kernel lost ~16us of DVE time to one of those after AR3).
"""

import sys

sys.path.insert(0, "/opt/trn_rl_repo")

import numpy as np
import ml_dtypes

import concourse.bacc as bacc
import concourse.mybir as mybir
import concourse.tile as tile
from concourse.bass_utils import run_bass_kernel_spmd

F32 = mybir.dt.float32
BF16 = mybir.dt.bfloat16
AX = mybir.AxisListType
OP = mybir.AluOpType
ACT = mybir.ActivationFunctionType

NCORES = 8
NI = 8                                   # images per core
C0, C1, C2 = 512, 256, 1024
H1, HW1 = 28, 784
H2, HW2 = 14, 196
PIX1, PIX2 = NI * HW1, NI * HW2          # 6272, 1568
K0, KM1, KM2 = C0 // 128, C1 // 128, C2 // 128   # 4, 2, 8
NCH = 392                                # pixel chunk (2 imgs at 14x14 / half img at 28x28)
PW = 29                                  # padded image width (top/left zero pad)
PIMG = PW * PW                           # 841
NG1 = 64 * HW1
NG2 = 64 * HW2
EPS = 1e-5
BF = ml_dtypes.bfloat16

TAPS = [(dy, dx) for dy in range(3) for dx in range(3)]

_CACHE = {}


def _ternarize_host(w):
    """fp32 ternarize matching the jax reference: returns (S, alpha)."""
    w = np.asarray(w, np.float32)
    absw = np.abs(w)
    delta = (0.7 * absw.mean(axis=(1, 2, 3), keepdims=True)).astype(np.float32)
    mask = (absw > delta).astype(np.float32)
    alpha = (absw * mask).sum(axis=(1, 2, 3)) / (mask.sum(axis=(1, 2, 3)) + 1e-8)
    sign = np.sign(w).astype(np.float32)
    return (sign * mask).astype(np.float32), alpha.astype(np.float32)


def build_program():
    nc = bacc.Bacc("TRN2", target_bir_lowering=False, debug=False,
                   num_devices=NCORES)

    x_d = nc.dram_tensor("x", [C0, PIX1], BF16, kind="ExternalInput")
    t1_d = nc.dram_tensor("t1", [C0, C1], BF16, kind="ExternalInput")
    t2_d = nc.dram_tensor("t2", [9, C1, C1], BF16, kind="ExternalInput")
    s2s_d = nc.dram_tensor("s2sum", [C1, C1], BF16, kind="ExternalInput")
    t3_d = nc.dram_tensor("t3", [C1, C2], BF16, kind="ExternalInput")
    dsw_d = nc.dram_tensor("dsw", [C0, C2], BF16, kind="ExternalInput")
    a1_d = nc.dram_tensor("alpha1", [C1], F32, kind="ExternalInput")
    a2_d = nc.dram_tensor("alpha2", [C1], F32, kind="ExternalInput")
    a3_d = nc.dram_tensor("alpha3", [C2], F32, kind="ExternalInput")
    bn1g_d = nc.dram_tensor("bn1g", [C0], F32, kind="ExternalInput")
    bn2g_d = nc.dram_tensor("bn2g", [C1], F32, kind="ExternalInput")
    bn2b_d = nc.dram_tensor("bn2b", [C1], F32, kind="ExternalInput")
    bn3g_d = nc.dram_tensor("bn3g", [C1], F32, kind="ExternalInput")
    bn3b_d = nc.dram_tensor("bn3b", [C1], F32, kind="ExternalInput")
    dsg_d = nc.dram_tensor("dsg", [C2], F32, kind="ExternalInput")
    dsb_d = nc.dram_tensor("dsb", [C2], F32, kind="ExternalInput")
    out_d = nc.dram_tensor("out", [C2, PIX2], BF16, kind="ExternalOutput")

    def colview(dram, m):
        # [m*128] dram vector -> SBUF [128, m] column tile access pattern
        return dram.ap().rearrange("(m p) -> p m", p=128)

    with tile.TileContext(nc) as tc:
        _build(nc, tc,
               x_d, t1_d, t2_d, s2s_d, t3_d, dsw_d,
               a1_d, a2_d, a3_d,
               bn1g_d, bn2g_d, bn2b_d, bn3g_d, bn3b_d, dsg_d, dsb_d,
               out_d, colview)

    nc.compile()
    return nc


def _build(nc, tc, x_d, t1_d, t2_d, s2s_d, t3_d, dsw_d,
           a1_d, a2_d, a3_d,
           bn1g_d, bn2g_d, bn2b_d, bn3g_d, bn3b_d, dsg_d, dsb_d,
           out_d, colview):
    from contextlib import ExitStack

    gctx = ExitStack()
    with gctx:
        dram = gctx.enter_context(tc.tile_pool(name="dram", bufs=1, space="DRAM"))
        sb = gctx.enter_context(tc.tile_pool(name="sb", bufs=1))
        sb_out = gctx.enter_context(tc.tile_pool(name="sb_out", bufs=4))
        ps_a = gctx.enter_context(tc.tile_pool(name="ps_a", bufs=4, space="PSUM"))
        ps_b = gctx.enter_context(tc.tile_pool(name="ps_b", bufs=4, space="PSUM"))

        # ------------- static loads -------------
        t1s = []
        for k in range(K0):
            tk = sb.tile([128, C1], BF16, name=f"t1_{k}")
            nc.sync.dma_start(tk[:], t1_d.ap()[k * 128:(k + 1) * 128, :])
            t1s.append(tk)
        dsws = []
        for k in range(K0):
            dk = sb.tile([128, C2], BF16, name=f"dsw_{k}")
            nc.sync.dma_start(dk[:], dsw_d.ap()[k * 128:(k + 1) * 128, :])
            dsws.append(dk)
        t2s = []
        for k in range(KM1):
            tk = sb.tile([128, 9, C1], BF16, name=f"t2_{k}")
            src = t2_d.ap()[:, k * 128:(k + 1) * 128, :].rearrange(
                "t p o -> p t o")
            nc.sync.dma_start(tk[:], src)
            t2s.append(tk)
        s2s = []
        for k in range(KM1):
            tk = sb.tile([128, C1], BF16, name=f"s2s_{k}")
            nc.sync.dma_start(tk[:], s2s_d.ap()[k * 128:(k + 1) * 128, :])
            s2s.append(tk)
        t3s = []
        for k in range(KM1):
            tk = sb.tile([128, C2], BF16, name=f"t3_{k}")
            nc.sync.dma_start(tk[:], t3_d.ap()[k * 128:(k + 1) * 128, :])
            t3s.append(tk)

        # per-channel parameter columns
        a1c = sb.tile([128, KM1], F32, name="a1c")
        nc.sync.dma_start(a1c[:], colview(a1_d, KM1))
        a2c = sb.tile([128, KM1], F32, name="a2c")
        nc.sync.dma_start(a2c[:], colview(a2_d, KM1))
        a3c = sb.tile([128, KM2], F32, name="a3c")
        nc.sync.dma_start(a3c[:], colview(a3_d, KM2))
        bn1g = sb.tile([128, K0], F32, name="bn1g")
        nc.sync.dma_start(bn1g[:], colview(bn1g_d, K0))
        bn2g = sb.tile([128, KM1], F32, name="bn2g")
        nc.sync.dma_start(bn2g[:], colview(bn2g_d, KM1))
        bn2b = sb.tile([128, KM1], F32, name="bn2b")
        nc.sync.dma_start(bn2b[:], colview(bn2b_d, KM1))
        bn3g = sb.tile([128, KM1], F32, name="bn3g")
        nc.sync.dma_start(bn3g[:], colview(bn3g_d, KM1))
        bn3b = sb.tile([128, KM1], F32, name="bn3b")
        nc.sync.dma_start(bn3b[:], colview(bn3b_d, KM1))
        dsg = sb.tile([128, KM2], F32, name="dsg")
        nc.sync.dma_start(dsg[:], colview(dsg_d, KM2))
        dsb = sb.tile([128, KM2], F32, name="dsb")
        nc.sync.dma_start(dsb[:], colview(dsb_d, KM2))

        # persistent activations
        xs = [sb.tile([128, PIX1], BF16, name=f"xs_{k}") for k in range(K0)]
        p1 = [sb.tile([128, NI * PIMG], BF16, name=f"p1_{m}")
              for m in range(KM1)]
        q = [sb.tile([128, PIX2], BF16, name=f"q_{m}") for m in range(KM2)]
        p2 = [sb.tile([128, PIX2], BF16, name=f"p2_{m}") for m in range(KM1)]
        dum = sb.tile([128, PIX2], BF16, name="dum")

        # stat tiles
        xbn = sb.tile([128, K0 * 16 * 6], F32, name="xbn")
        xagg = sb.tile([128, K0 * 2], F32, name="xagg")
        st1 = sb.tile([128, 2 * K0], F32, name="st1")
        g1 = sb.tile([128, 2 * K0], F32, name="g1")
        qsum_c = sb.tile([128, KM2 * 4], F32, name="qsum_c")
        p1bn = sb.tile([128, KM1 * 16 * 6], F32, name="p1bn")
        p1agg = sb.tile([128, KM1 * 2], F32, name="p1agg")
        st2 = sb.tile([128, 20], F32, name="st2")
        g2 = sb.tile([128, 20], F32, name="g2")
        p2bn = sb.tile([128, KM1 * 4 * 6], F32, name="p2bn")
        p2agg = sb.tile([128, KM1 * 2], F32, name="p2agg")
        p2s_c = sb.tile([128, KM1 * 4], F32, name="p2s_c")
        st3 = sb.tile([128, 4], F32, name="st3")
        g3 = sb.tile([128, 4], F32, name="g3")

        # x streamed in per image pair (8 chunks per pair across k)
        for p in range(4):
            for k in range(K0):
                nc.sync.dma_start(
                    xs[k][:, p * 2 * HW1:(p + 1) * 2 * HW1],
                    x_d.ap()[k * 128:(k + 1) * 128,
                             p * 2 * HW1:(p + 1) * 2 * HW1])

        # ================= phase A: x stats + ds conv =================
        for p in range(4):
            for k in range(K0):
                for g in range(4):
                    slot = k * 16 + p * 4 + g
                    nc.vector.bn_stats(
                        xbn[:, slot * 6:(slot + 1) * 6],
                        xs[k][:, p * 1568 + g * 392:p * 1568 + (g + 1) * 392])
            for m in range(KM2):
                pmm = ps_a.tile([128, NCH], F32, tag="mm",
                                name=f"psds_{p}_{m}")
                for k in range(K0):
                    rhs = xs[k][:, p * 1568:(p + 1) * 1568].rearrange(
                        "p (i ay by ax bx) -> p i by bx ay ax",
                        i=2, ay=14, by=2, ax=14, bx=2)[:, :, 0, 0, :, :]
                    nc.tensor.matmul(
                        pmm[:], dsws[k][:, m * 128:(m + 1) * 128],
                        rhs, start=(k == 0), stop=(k == K0 - 1))
                dst = q[m][:, p * NCH:(p + 1) * NCH]
                acc = qsum_c[:, m * 4 + p:m * 4 + p + 1]
                if m % 2 == 0:
                    nc.vector.tensor_scalar(dst, pmm[:], 1.0, 0.0,
                                            OP.mult, OP.add, accum_out=acc)
                else:
                    nc.scalar.activation(dst, pmm[:], ACT.Copy, accum_out=acc)

        # x stat aggregation -> st1 -> AR1
        xbnv = xbn[:].rearrange("p (kg s) -> p kg s", s=6)
        for k in range(K0):
            nc.vector.bn_aggr(xagg[:, k * 2:k * 2 + 2],
                              xbnv[:, k * 16:(k + 1) * 16, :])
        xaggv = xagg[:].rearrange("p (k two) -> p k two", two=2)
        mcols = xaggv[:, :, 0]
        vcols = xaggv[:, :, 1]
        msq = sb.tile([128, K0], F32, name="msq")
        nc.vector.tensor_tensor(msq[:], mcols, mcols, OP.mult)
        nc.vector.tensor_scalar(st1[:, 0:K0], mcols, float(PIX1), None, OP.mult)
        nc.vector.tensor_tensor(msq[:], msq[:], vcols, OP.add)
        nc.vector.tensor_scalar(st1[:, K0:2 * K0], msq[:], float(PIX1), None,
                                OP.mult)
        ar1_in = dram.tile([128, 2 * K0], F32, name="ar1_in")
        ar1_out = dram.tile([128, 2 * K0], F32, name="ar1_out")
        nc.sync.dma_start(ar1_in[:], st1[:])
        nc.gpsimd.collective_compute(
            "AllReduce", OP.add, replica_groups=[list(range(NCORES))],
            ins=[ar1_in.opt()], outs=[ar1_out.opt()])
        nc.sync.dma_start(g1[:], ar1_out[:])

        # q sumsq (ACT) + q sum reduce (DVE) — overlap AR1
        for m in range(KM2):
            nc.scalar.activation(dum[:], q[m][:], ACT.Square,
                                 accum_out=st2[:, 12 + m:13 + m])
            nc.vector.reduce_sum(st2[:, 4 + m:5 + m],
                                 qsum_c[:, m * 4:(m + 1) * 4], axis=AX.X)

        # ---- BN1 affine -> fold into t1 ----
        mean_x = sb.tile([128, K0], F32, name="mean_x")
        var_x = sb.tile([128, K0], F32, name="var_x")
        a1f = sb.tile([128, K0], F32, name="a1f")
        tmp_k0 = sb.tile([128, K0], F32, name="tmp_k0")
        nc.vector.tensor_scalar(mean_x[:], g1[:, 0:K0], 1.0 / NG1, None, OP.mult)
        nc.vector.tensor_tensor(tmp_k0[:], mean_x[:], mean_x[:], OP.mult)
        nc.vector.tensor_scalar(var_x[:], g1[:, K0:2 * K0], 1.0 / NG1, None,
                                OP.mult)
        nc.vector.tensor_tensor(var_x[:], var_x[:], tmp_k0[:], OP.subtract)

        def rsqrt_cols(dst, var_ap, gamma_ap, extra_mul=None):
            cols = dst.shape[1]
            tmp = sb.tile([128, cols], F32, tag="rsq_tmp",
                          name=f"rsq_{dst.tensor.name}")
            nc.vector.tensor_scalar(tmp[:], var_ap, EPS, None, OP.add)
            nc.vector.reciprocal(tmp[:], tmp[:])
            nc.scalar.sqrt(tmp[:], tmp[:])
            nc.vector.tensor_tensor(dst, tmp[:], gamma_ap, OP.mult)
            if extra_mul is not None:
                nc.vector.tensor_tensor(dst, dst, extra_mul, OP.mult)

        rsqrt_cols(a1f[:], var_x[:], bn1g[:])
        for k in range(K0):
            nc.vector.tensor_scalar(t1s[k][:], t1s[k][:], a1f[:, k:k + 1],
                                    None, OP.mult)

        # ================= conv1 (raw x @ folded t1) =================
        for m in range(KM1):
            for img in range(NI):
                for half in range(2):
                    pmm = ps_a.tile([128, NCH], F32, tag="mm",
                                    name=f"psc1_{m}_{img}_{half}")
                    for k in range(K0):
                        nc.tensor.matmul(
                            pmm[:], t1s[k][:, m * 128:(m + 1) * 128],
                            xs[k][:, img * HW1 + half * NCH:
                                  img * HW1 + (half + 1) * NCH],
                            start=(k == 0), stop=(k == K0 - 1))
                    dst = p1[m][:, img * PIMG:(img + 1) * PIMG].rearrange(
                        "p (r c) -> p r c", r=PW, c=PW)[
                        :, 1 + half * 14:1 + (half + 1) * 14, 1:PW]
                    nc.scalar.activation(
                        dst, pmm[:].rearrange("p (r c) -> p r c", r=14, c=28),
                        ACT.Copy)
                    slot = m * 16 + img * 2 + half
                    nc.vector.bn_stats(
                        p1bn[:, slot * 6:(slot + 1) * 6], pmm[:])

        # p1 stat aggregation -> st2 -> AR2
        for m in range(KM1):
            groups = p1bn[:, m * 96:(m + 1) * 96].rearrange(
                "p (g s) -> p g s", s=6)
            nc.vector.bn_aggr(p1agg[:, m * 2:m * 2 + 2], groups)
        pav = p1agg[:].rearrange("p (m two) -> p m two", two=2)
        pmsq = sb.tile([128, KM1], F32, name="pmsq")
        nc.vector.tensor_tensor(pmsq[:], pav[:, :, 0], pav[:, :, 0], OP.mult)
        nc.vector.tensor_scalar(st2[:, 0:KM1], pav[:, :, 0], float(PIX1),
                                None, OP.mult)
        nc.vector.tensor_tensor(pmsq[:], pmsq[:], pav[:, :, 1], OP.add)
        nc.vector.tensor_scalar(st2[:, KM1:2 * KM1], pmsq[:], float(PIX1),
                                None, OP.mult)
        ar2_in = dram.tile([128, 20], F32, name="ar2_in")
        ar2_out = dram.tile([128, 20], F32, name="ar2_out")
        nc.sync.dma_start(ar2_in[:], st2[:])
        nc.gpsimd.collective_compute(
            "AllReduce", OP.add, replica_groups=[list(range(NCORES))],
            ins=[ar2_in.opt()], outs=[ar2_out.opt()])
        nc.sync.dma_start(g2[:], ar2_out[:])

        # ---- post-AR2 column math ----
        mean_p1 = sb.tile([128, KM1], F32, name="mean_p1")
        var_p1 = sb.tile([128, KM1], F32, name="var_p1")
        a2f = sb.tile([128, KM1], F32, name="a2f")
        b2f = sb.tile([128, KM1], F32, name="b2f")
        v2 = sb.tile([128, KM1], F32, name="v2")
        b2fb = sb.tile([128, KM1], BF16, name="b2fb")
        tmp_m1 = sb.tile([128, KM1], F32, name="tmp_m1")
        nc.vector.tensor_scalar(mean_p1[:], g2[:, 0:2], 1.0 / NG1, None, OP.mult)
        nc.vector.tensor_scalar(var_p1[:], g2[:, 2:4], 1.0 / NG1, None, OP.mult)
        nc.vector.tensor_tensor(tmp_m1[:], mean_p1[:], mean_p1[:], OP.mult)
        nc.vector.tensor_tensor(var_p1[:], var_p1[:], tmp_m1[:], OP.subtract)
        nc.vector.tensor_tensor(tmp_m1[:], a1c[:], a1c[:], OP.mult)
        nc.vector.tensor_tensor(var_p1[:], var_p1[:], tmp_m1[:], OP.mult)
        rsqrt_cols(a2f[:], var_p1[:], bn2g[:], extra_mul=a1c[:])
        nc.vector.tensor_tensor(tmp_m1[:], a2f[:], mean_p1[:], OP.mult)
        nc.vector.tensor_tensor(b2f[:], bn2b[:], tmp_m1[:], OP.subtract)
        nc.vector.tensor_copy(b2fb[:], b2f[:])
        # v2 = -b2f/a2f = mean_p1 - bn2b/a2f
        nc.vector.reciprocal(tmp_m1[:], a2f[:])
        nc.vector.tensor_tensor(tmp_m1[:], bn2b[:], tmp_m1[:], OP.mult)
        nc.vector.tensor_tensor(v2[:], mean_p1[:], tmp_m1[:], OP.subtract)

        # ds BN affine (uses AR2 q stats)
        mean_q = sb.tile([128, KM2], F32, name="mean_q")
        var_q = sb.tile([128, KM2], F32, name="var_q")
        aq = sb.tile([128, KM2], F32, name="aq")
        bq = sb.tile([128, KM2], F32, name="bq")
        tmp_m2 = sb.tile([128, KM2], F32, name="tmp_m2")
        nc.vector.tensor_scalar(mean_q[:], g2[:, 4:12], 1.0 / NG2, None, OP.mult)
        nc.vector.tensor_scalar(var_q[:], g2[:, 12:20], 1.0 / NG2, None, OP.mult)
        nc.vector.tensor_tensor(tmp_m2[:], mean_q[:], mean_q[:], OP.mult)
        nc.vector.tensor_tensor(var_q[:], var_q[:], tmp_m2[:], OP.subtract)
        rsqrt_cols(aq[:], var_q[:], dsg[:])
        nc.vector.tensor_tensor(tmp_m2[:], aq[:], mean_q[:], OP.mult)
        nc.vector.tensor_tensor(bq[:], dsb[:], tmp_m2[:], OP.subtract)

        # fold a2f into t2 weights; fill p1 pads with v2
        for k in range(KM1):
            nc.vector.tensor_scalar(
                t2s[k][:].rearrange("p t o -> p (t o)"),
                t2s[k][:].rearrange("p t o -> p (t o)"),
                a2f[:, k:k + 1], None, OP.mult)
        for m in range(KM1):
            pv = p1[m][:].rearrange("p (i r c) -> p i r c", i=NI, r=PW, c=PW)
            nc.vector.tensor_copy(
                pv[:, :, 0, :], v2[:, m:m + 1].broadcast_to([128, NI, PW]))
            nc.vector.tensor_copy(
                pv[:, :, 1:PW, 0], v2[:, m:m + 1].broadcast_to([128, NI, 28]))

        # T[o] = sum_i S2sum[i,o] * b2f[i]  (border-free BN2 shift)
        Tc = sb.tile([128, KM1], F32, name="Tc")
        for m in range(KM1):
            tps = ps_b.tile([128, NCH], F32, tag="c2", name=f"tps_{m}")
            for k in range(KM1):
                nc.tensor.matmul(tps[:, 0:1], s2s[k][:, m * 128:(m + 1) * 128],
                                 b2fb[:, k:k + 1],
                                 start=(k == 0), stop=(k == KM1 - 1))
            nc.vector.tensor_copy(Tc[:, m:m + 1], tps[:, 0:1])

        # ================= conv2: 3x3 s2 p1 from padded p1 =================
        p1v = [p1[k][:].rearrange("p (i r c) -> p i r c", i=NI, r=PW, c=PW)
               for k in range(KM1)]
        for m in range(KM1):
            pmms = [ps_b.tile([128, NCH], F32, tag="c2",
                              name=f"psc2_{m}_{ip}") for ip in range(4)]
            for t, (dy, dx) in enumerate(TAPS):
                for k in range(KM1):
                    for ip in range(4):
                        rhs = p1v[k][:, 2 * ip:2 * ip + 2,
                                     dy:dy + 27:2, dx:dx + 27:2]
                        nc.tensor.matmul(
                            pmms[ip][:],
                            t2s[k][:, t, m * 128:(m + 1) * 128],
                            rhs, start=(t == 0 and k == 0),
                            stop=(t == 8 and k == KM1 - 1))
            for ip in range(4):
                nc.vector.tensor_scalar(
                    p2[m][:, ip * NCH:(ip + 1) * NCH], pmms[ip][:],
                    1.0, Tc[:, m:m + 1], OP.mult, OP.add,
                    accum_out=p2s_c[:, m * 4 + ip:m * 4 + ip + 1])
                slot = m * 4 + ip
                nc.vector.bn_stats(p2bn[:, slot * 6:(slot + 1) * 6],
                                   p2[m][:, ip * NCH:(ip + 1) * NCH])

        # p2 stats -> st3 -> AR3
        for m in range(KM1):
            nc.vector.reduce_sum(st3[:, m:m + 1],
                                 p2s_c[:, m * 4:(m + 1) * 4], axis=AX.X)
            nc.vector.bn_aggr(
                p2agg[:, m * 2:m * 2 + 2],
                p2bn[:, m * 24:(m + 1) * 24].rearrange("p (g s) -> p g s", s=6))
        p2av = p2agg[:].rearrange("p (m two) -> p m two", two=2)
        p2msq = sb.tile([128, KM1], F32, name="p2msq")
        nc.vector.tensor_tensor(p2msq[:], p2av[:, :, 0], p2av[:, :, 0], OP.mult)
        nc.vector.tensor_tensor(p2msq[:], p2msq[:], p2av[:, :, 1], OP.add)
        nc.vector.tensor_scalar(st3[:, 2:4], p2msq[:], float(PIX2), None,
                                OP.mult)
        ar3_in = dram.tile([128, 4], F32, name="ar3_in")
        ar3_out = dram.tile([128, 4], F32, name="ar3_out")
        nc.sync.dma_start(ar3_in[:], st3[:])
        nc.gpsimd.collective_compute(
            "AllReduce", OP.add, replica_groups=[list(range(NCORES))],
            ins=[ar3_in.opt()], outs=[ar3_out.opt()])
        nc.sync.dma_start(g3[:], ar3_out[:])

        # shortcut affine in place on q — overlaps AR3 (needs only AR2 stats)
        for m in range(KM2):
            nc.vector.tensor_scalar(q[m][:], q[m][:], aq[:, m:m + 1],
                                    bq[:, m:m + 1], OP.mult, OP.add)

        # ---- BN3 affine (alpha2-corrected) -> xn3 in place on p2 ----
        mean_p2 = sb.tile([128, KM1], F32, name="mean_p2")
        var_p2 = sb.tile([128, KM1], F32, name="var_p2")
        a3f = sb.tile([128, KM1], F32, name="a3f")
        b3f = sb.tile([128, KM1], F32, name="b3f")
        tmp_m3 = sb.tile([128, KM1], F32, name="tmp_m3")
        nc.vector.tensor_scalar(mean_p2[:], g3[:, 0:2], 1.0 / NG2, None, OP.mult)
        nc.vector.tensor_scalar(var_p2[:], g3[:, 2:4], 1.0 / NG2, None, OP.mult)
        nc.vector.tensor_tensor(tmp_m3[:], mean_p2[:], mean_p2[:], OP.mult)
        nc.vector.tensor_tensor(var_p2[:], var_p2[:], tmp_m3[:], OP.subtract)
        nc.vector.tensor_tensor(tmp_m3[:], a2c[:], a2c[:], OP.mult)
        nc.vector.tensor_tensor(var_p2[:], var_p2[:], tmp_m3[:], OP.mult)
        rsqrt_cols(a3f[:], var_p2[:], bn3g[:], extra_mul=a2c[:])
        nc.vector.tensor_tensor(tmp_m3[:], a3f[:], mean_p2[:], OP.mult)
        nc.vector.tensor_tensor(b3f[:], bn3b[:], tmp_m3[:], OP.subtract)
        for m in range(KM1):
            nc.vector.tensor_scalar(p2[m][:], p2[m][:], a3f[:, m:m + 1],
                                    b3f[:, m:m + 1], OP.mult, OP.add)

        # ================= conv3 + residual fuse + store =================
        for m in range(KM2):
            for j in range(4):
                pmm = ps_a.tile([128, NCH], F32, tag="mm",
                                name=f"psc3_{m}_{j}")
                for k in range(KM1):
                    nc.tensor.matmul(
                        pmm[:], t3s[k][:, m * 128:(m + 1) * 128],
                        p2[k][:, j * NCH:(j + 1) * NCH],
                        start=(k == 0), stop=(k == KM1 - 1))
                out_t = sb_out.tile([128, NCH], BF16, tag="out",
                                    name=f"out_{m}_{j}")
                nc.vector.scalar_tensor_tensor(
                    out_t[:], pmm[:], a3c[:, m:m + 1],
                    q[m][:, j * NCH:(j + 1) * NCH], OP.mult, OP.add)
                nc.sync.dma_start(
                    out_d.ap()[m * 128:(m + 1) * 128,
                               j * NCH:(j + 1) * NCH],
                    out_t[:])


def _prep_host(inputs):
    """Host-side: shard x, fold weights, build per-core in_maps."""
    x = np.asarray(inputs["x"], np.float32)          # [64, 512, 28, 28]
    w1 = np.asarray(inputs["w1"], np.float32)
    w2 = np.asarray(inputs["w2"], np.float32)
    w3 = np.asarray(inputs["w3"], np.float32)
    ds_w = np.asarray(inputs["ds_w"], np.float32)

    s1, al1 = _ternarize_host(w1)    # [256,512,1,1]
    s2, al2 = _ternarize_host(w2)    # [256,256,3,3]
    s3, al3 = _ternarize_host(w3)    # [1024,256,1,1]

    t1 = np.ascontiguousarray(s1[:, :, 0, 0].T).astype(BF)       # [512, 256]
    # t2[t, i, o] = s2[o, i, ky, kx], t = ky*3+kx
    t2 = np.ascontiguousarray(
        s2.transpose(2, 3, 1, 0).reshape(9, C1, C1)).astype(BF)
    s2sum = t2.astype(np.float32).sum(axis=0).astype(BF)         # [256, 256]
    t3 = np.ascontiguousarray(s3[:, :, 0, 0].T).astype(BF)       # [256, 1024]
    dsw = np.ascontiguousarray(ds_w[:, :, 0, 0].T).astype(BF)    # [512, 1024]

    common = dict(
        t1=t1, t2=t2, s2sum=s2sum, t3=t3, dsw=dsw,
        alpha1=al1, alpha2=al2, alpha3=al3,
        bn1g=np.asarray(inputs["bn1_g"], np.float32),
        bn2g=np.asarray(inputs["bn2_g"], np.float32),
        bn2b=np.asarray(inputs["bn2_b"], np.float32),
        bn3g=np.asarray(inputs["bn3_g"], np.float32),
        bn3b=np.asarray(inputs["bn3_b"], np.float32),
        dsg=np.asarray(inputs["ds_bn_g"], np.float32),
        dsb=np.asarray(inputs["ds_bn_b"], np.float32),
    )

    in_maps = []
    for c in range(NCORES):
        xc = x[c * NI:(c + 1) * NI]                      # [8, 512, 28, 28]
        xc = np.ascontiguousarray(
            xc.transpose(1, 0, 2, 3).reshape(C0, PIX1)).astype(BF)
        in_maps.append({"x": xc, **common})
    return in_maps


def kernel(**inputs):
    if "nc" not in _CACHE:
        _CACHE["nc"] = build_program()
    nc = _CACHE["nc"]

    in_maps = _prep_host(inputs)
    try:
        res = run_bass_kernel_spmd(nc, in_maps, core_ids=list(range(NCORES)))
    except Exception:
        # transient device state (e.g. a previous crashed run) usually
        # clears on retry
        res = run_bass_kernel_spmd(nc, in_maps, core_ids=list(range(NCORES)))

    out = np.empty((64, C2, H2, H2), np.float32)
    for c in range(NCORES):
        oc = np.asarray(res.results[c]["out"]).astype(np.float32)
        oc = oc.reshape(C2, NI, H2, H2)
        out[c * NI:(c + 1) * NI] = oc.transpose(1, 0, 2, 3)
    return out


# revision 32
# speedup vs baseline: 1.2441x; 1.0707x over previous
"""Trainium2 Bass kernel for nn_BottleneckR (pre-activation ternary bottleneck).

Reference (batch 64):
  h  = conv1x1(BN1(x), tern(w1))            [64,256,28,28]
  h  = conv3x3s2p1(BN2(h), tern(w2))        [64,256,14,14]
  h  = conv1x1(BN3(h), tern(w3))            [64,1024,14,14]
  sc = BN_ds(conv1x1s2(x, ds_w))            [64,1024,14,14]
  out = h + sc

Strategy: data-parallel over batch on 8 NeuronCores (8 images/core).
Exact sync-BN via 3 small AllReduces of per-channel sum/sumsq.

Differences vs the v1 kernel (306us):
 - bf16 end to end: x is converted to bf16 on the host (halves the HBM
   stream), all matmuls run bf16 at full PE rate, DVE elementwise ops get
   the 2x 16-bit mode. Ternary sign matrices are exact in bf16.
 - x stays resident in SBUF (one HBM read instead of two).
 - p1 is stored in a 29x29 zero-padded per-image layout; conv2's 9 taps
   read it directly with stride-2 access patterns (no phase-split scatter,
   no shifted-plane copies).
 - BN2 is folded into conv2 instead of materialized: the scale a2f folds
   into the t2 weights (per input channel), the shift b2f is handled by
   adding T[o] = sum_i S2sum[i,o]*b2f[i] at the psum->p2 copy while the
   pad cells hold -b2f/a2f (so border taps contribute zero shift, exactly
   matching zero padding after BN).
 - AR1 is triggered as soon as the x statistics are done; the ds-conv
   tail, q sum-of-squares and all q/p1 copies overlap the collective.
 - q's BN affine runs during AR3's latency window.
 - One persistent tile pool: no mid-kernel pool-exit barriers (the v1
   kernel lost ~16us of DVE time to one of those after AR3).
"""

import sys

sys.path.insert(0, "/opt/trn_rl_repo")

import numpy as np
import ml_dtypes

import concourse.bacc as bacc
import concourse.mybir as mybir
import concourse.tile as tile
from concourse.bass_utils import run_bass_kernel_spmd

F32 = mybir.dt.float32
BF16 = mybir.dt.bfloat16
AX = mybir.AxisListType
OP = mybir.AluOpType
ACT = mybir.ActivationFunctionType

NCORES = 8
NI = 8                                   # images per core
C0, C1, C2 = 512, 256, 1024
H1, HW1 = 28, 784
H2, HW2 = 14, 196
PIX1, PIX2 = NI * HW1, NI * HW2          # 6272, 1568
K0, KM1, KM2 = C0 // 128, C1 // 128, C2 // 128   # 4, 2, 8
NCH = 392                                # pixel chunk (2 imgs at 14x14 / half img at 28x28)
PW = 29                                  # padded image width (top/left zero pad)
PIMG = PW * PW                           # 841
NG1 = 64 * HW1
NG2 = 64 * HW2
EPS = 1e-5
BF = ml_dtypes.bfloat16

TAPS = [(dy, dx) for dy in range(3) for dx in range(3)]

_CACHE = {}


def _ternarize_host(w):
    """fp32 ternarize matching the jax reference: returns (S, alpha)."""
    w = np.asarray(w, np.float32)
    absw = np.abs(w)
    delta = (0.7 * absw.mean(axis=(1, 2, 3), keepdims=True)).astype(np.float32)
    mask = (absw > delta).astype(np.float32)
    alpha = (absw * mask).sum(axis=(1, 2, 3)) / (mask.sum(axis=(1, 2, 3)) + 1e-8)
    sign = np.sign(w).astype(np.float32)
    return (sign * mask).astype(np.float32), alpha.astype(np.float32)


def build_program():
    nc = bacc.Bacc("TRN2", target_bir_lowering=False, debug=False,
                   num_devices=NCORES)

    x_d = nc.dram_tensor("x", [C0, PIX1], BF16, kind="ExternalInput")
    t1_d = nc.dram_tensor("t1", [C0, C1], BF16, kind="ExternalInput")
    # t2 pre-arranged on host: [k, i_local, t*C1+o]
    t2_d = nc.dram_tensor("t2", [KM1, 128, 9 * C1], BF16, kind="ExternalInput")
    s2s_d = nc.dram_tensor("s2sum", [C1, C1], BF16, kind="ExternalInput")
    t3_d = nc.dram_tensor("t3", [C1, C2], BF16, kind="ExternalInput")
    dsw_d = nc.dram_tensor("dsw", [C0, C2], BF16, kind="ExternalInput")
    # all per-channel params pre-arranged on host as [128, n] column tiles
    # (one contiguous DMA row per partition instead of thousands of 4B
    # descriptors): [a1(2) a2(2) a3(8) bn1g(4) bn2g(2) bn2b(2) bn3g(2)
    # bn3b(2) dsg(8) dsb(8)] = 40 cols
    par_d = nc.dram_tensor("par", [128, 40], F32, kind="ExternalInput")
    out_d = nc.dram_tensor("out", [C2, PIX2], BF16, kind="ExternalOutput")

    with tile.TileContext(nc) as tc:
        _build(nc, tc, x_d, t1_d, t2_d, s2s_d, t3_d, dsw_d, par_d, out_d)

    nc.compile()
    return nc


def _build(nc, tc, x_d, t1_d, t2_d, s2s_d, t3_d, dsw_d, par_d, out_d):
    from contextlib import ExitStack

    gctx = ExitStack()
    with gctx:
        dram = gctx.enter_context(tc.tile_pool(name="dram", bufs=1, space="DRAM"))
        sb = gctx.enter_context(tc.tile_pool(name="sb", bufs=1))
        sb_out = gctx.enter_context(tc.tile_pool(name="sb_out", bufs=4))
        ps_a = gctx.enter_context(tc.tile_pool(name="ps_a", bufs=4, space="PSUM"))
        ps_b = gctx.enter_context(tc.tile_pool(name="ps_b", bufs=4, space="PSUM"))

        # ------------- static loads -------------
        t1s = []
        for k in range(K0):
            tk = sb.tile([128, C1], BF16, name=f"t1_{k}")
            nc.sync.dma_start(tk[:], t1_d.ap()[k * 128:(k + 1) * 128, :])
            t1s.append(tk)
        dsws = []
        for k in range(K0):
            dk = sb.tile([128, C2], BF16, name=f"dsw_{k}")
            nc.sync.dma_start(dk[:], dsw_d.ap()[k * 128:(k + 1) * 128, :])
            dsws.append(dk)
        t2s = []
        for k in range(KM1):
            tk = sb.tile([128, 9, C1], BF16, name=f"t2_{k}")
            nc.sync.dma_start(
                tk[:].rearrange("p t o -> p (t o)"), t2_d.ap()[k])
            t2s.append(tk)
        s2s = []
        for k in range(KM1):
            tk = sb.tile([128, C1], BF16, name=f"s2s_{k}")
            nc.sync.dma_start(tk[:], s2s_d.ap()[k * 128:(k + 1) * 128, :])
            s2s.append(tk)
        t3s = []
        for k in range(KM1):
            tk = sb.tile([128, C2], BF16, name=f"t3_{k}")
            nc.sync.dma_start(tk[:], t3_d.ap()[k * 128:(k + 1) * 128, :])
            t3s.append(tk)

        # per-channel parameter columns (one DMA, host pre-arranged)
        par = sb.tile([128, 40], F32, name="par")
        nc.sync.dma_start(par[:], par_d.ap())
        a1c = par[:, 0:2]
        a2c = par[:, 2:4]
        a3c = par[:, 4:12]
        bn1g = par[:, 12:16]
        bn2g = par[:, 16:18]
        bn2b = par[:, 18:20]
        bn3g = par[:, 20:22]
        bn3b = par[:, 22:24]
        dsg = par[:, 24:32]
        dsb = par[:, 32:40]

        # persistent activations
        xs = [sb.tile([128, PIX1], BF16, name=f"xs_{k}") for k in range(K0)]
        p1 = [sb.tile([128, NI * PIMG], BF16, name=f"p1_{m}")
              for m in range(KM1)]
        q = [sb.tile([128, PIX2], BF16, name=f"q_{m}") for m in range(KM2)]
        p2 = [sb.tile([128, PIX2], BF16, name=f"p2_{m}") for m in range(KM1)]
        dum = sb.tile([128, PIX2], BF16, name="dum")     # ACT scratch
        dum2 = sb.tile([128, PIX2], BF16, name="dum2")   # Pool scratch
        dumd = sb.tile([128, PIX2], BF16, name="dumd")   # DVE scratch

        # stat tiles
        xsum_c = sb.tile([128, K0 * 4], F32, name="xsum_c")
        xsq_c = sb.tile([128, K0 * 4], F32, name="xsq_c")
        st1 = sb.tile([128, 2 * K0], F32, name="st1")
        g1 = sb.tile([128, 2 * K0], F32, name="g1")
        qsum_c = sb.tile([128, KM2 * 4], F32, name="qsum_c")
        p1s_c = sb.tile([128, KM1 * 16], F32, name="p1s_c")
        p1q_c = sb.tile([128, KM1 * 16], F32, name="p1q_c")
        st2 = sb.tile([128, 20], F32, name="st2")
        g2 = sb.tile([128, 20], F32, name="g2")
        p2s_c = sb.tile([128, KM1 * 4], F32, name="p2s_c")
        p2q_c = sb.tile([128, KM1 * 4], F32, name="p2q_c")
        st3 = sb.tile([128, 4], F32, name="st3")
        g3 = sb.tile([128, 4], F32, name="g3")

        # x streamed in per image pair (8 chunks per pair across k)
        for p in range(4):
            for k in range(K0):
                nc.sync.dma_start(
                    xs[k][:, p * 2 * HW1:(p + 1) * 2 * HW1],
                    x_d.ap()[k * 128:(k + 1) * 128,
                             p * 2 * HW1:(p + 1) * 2 * HW1])

        # ================= phase A: x stats + ds conv =================
        # x statistics as pure accumulator passes on DVE (sums, bf16 2x)
        # and ACT/DVE (squares); Pool cannot do arithmetic.
        for p in range(4):
            for k in range(K0):
                chunk = xs[k][:, p * 1568:(p + 1) * 1568]
                c = k * 4 + p
                nc.vector.tensor_scalar(
                    dumd[:], chunk, 1.0, 0.0, OP.mult, OP.add,
                    accum_out=xsum_c[:, c:c + 1])
                if c < 11:
                    nc.scalar.activation(
                        dum[:], chunk, ACT.Square,
                        accum_out=xsq_c[:, c:c + 1])
                else:
                    nc.vector.scalar_tensor_tensor(
                        dumd[:], chunk, 1.0, chunk, OP.mult, OP.mult,
                        accum_out=xsq_c[:, c:c + 1])
            for m in range(KM2):
                pool = ps_a if m % 2 == 0 else ps_b
                tagn = "mm" if m % 2 == 0 else "c2"
                pmm = pool.tile([128, NCH], F32, tag=tagn,
                                name=f"psds_{p}_{m}")
                for k in range(K0):
                    rhs = xs[k][:, p * 1568:(p + 1) * 1568].rearrange(
                        "p (i ay by ax bx) -> p i by bx ay ax",
                        i=2, ay=14, by=2, ax=14, bx=2)[:, :, 0, 0, :, :]
                    nc.tensor.matmul(
                        pmm[:], dsws[k][:, m * 128:(m + 1) * 128],
                        rhs, start=(k == 0), stop=(k == K0 - 1))
                dst = q[m][:, p * NCH:(p + 1) * NCH]
                acc = qsum_c[:, m * 4 + p:m * 4 + p + 1]
                if m % 2 == 0:
                    nc.vector.tensor_scalar(dst, pmm[:], 1.0, 0.0,
                                            OP.mult, OP.add, accum_out=acc)
                else:
                    nc.scalar.activation(dst, pmm[:], ACT.Copy, accum_out=acc)

        # x stat reduction -> st1 -> AR1
        for k in range(K0):
            nc.vector.reduce_sum(st1[:, k:k + 1],
                                 xsum_c[:, k * 4:(k + 1) * 4], axis=AX.X)
            nc.vector.reduce_sum(st1[:, K0 + k:K0 + k + 1],
                                 xsq_c[:, k * 4:(k + 1) * 4], axis=AX.X)
        ar1_in = dram.tile([128, 2 * K0], F32, name="ar1_in")
        ar1_out = dram.tile([128, 2 * K0], F32, name="ar1_out")
        nc.sync.dma_start(ar1_in[:], st1[:])
        nc.gpsimd.collective_compute(
            "AllReduce", OP.add, replica_groups=[list(range(NCORES))],
            ins=[ar1_in.opt()], outs=[ar1_out.opt()])
        nc.sync.dma_start(g1[:], ar1_out[:])

        # q sumsq (ACT) + q sum reduce (DVE) — overlap AR1
        for m in range(KM2):
            nc.scalar.activation(dum[:], q[m][:], ACT.Square,
                                 accum_out=st2[:, 12 + m:13 + m])
            nc.vector.reduce_sum(st2[:, 4 + m:5 + m],
                                 qsum_c[:, m * 4:(m + 1) * 4], axis=AX.X)

        # ---- BN1 affine -> fold into t1 ----
        mean_x = sb.tile([128, K0], F32, name="mean_x")
        var_x = sb.tile([128, K0], F32, name="var_x")
        a1f = sb.tile([128, K0], F32, name="a1f")
        tmp_k0 = sb.tile([128, K0], F32, name="tmp_k0")
        nc.vector.tensor_scalar(mean_x[:], g1[:, 0:K0], 1.0 / NG1, None, OP.mult)
        nc.vector.tensor_tensor(tmp_k0[:], mean_x[:], mean_x[:], OP.mult)
        nc.vector.tensor_scalar(var_x[:], g1[:, K0:2 * K0], 1.0 / NG1, None,
                                OP.mult)
        nc.vector.tensor_tensor(var_x[:], var_x[:], tmp_k0[:], OP.subtract)

        def rsqrt_cols(dst, var_ap, gamma_ap, extra_mul=None):
            cols = dst.shape[1]
            tmp = sb.tile([128, cols], F32, tag="rsq_tmp",
                          name=f"rsq_{dst.tensor.name}")
            nc.vector.tensor_scalar(tmp[:], var_ap, EPS, None, OP.add)
            nc.vector.reciprocal(tmp[:], tmp[:])
            nc.scalar.sqrt(tmp[:], tmp[:])
            nc.vector.tensor_tensor(dst, tmp[:], gamma_ap, OP.mult)
            if extra_mul is not None:
                nc.vector.tensor_tensor(dst, dst, extra_mul, OP.mult)

        rsqrt_cols(a1f[:], var_x[:], bn1g)
        for k in range(K0):
            nc.vector.tensor_scalar(t1s[k][:], t1s[k][:], a1f[:, k:k + 1],
                                    None, OP.mult)

        # ================= conv1 (raw x @ folded t1) =================
        # psum->p1 copies alternate DVE/ACT (both with sum accumulators);
        # sumsq per chunk on Pool (scalar_tensor_tensor square).
        for m in range(KM1):
            for img in range(NI):
                for half in range(2):
                    pmm = ps_a.tile([128, NCH], F32, tag="mm",
                                    name=f"psc1_{m}_{img}_{half}")
                    for k in range(K0):
                        nc.tensor.matmul(
                            pmm[:], t1s[k][:, m * 128:(m + 1) * 128],
                            xs[k][:, img * HW1 + half * NCH:
                                  img * HW1 + (half + 1) * NCH],
                            start=(k == 0), stop=(k == K0 - 1))
                    dst = p1[m][:, img * PIMG:(img + 1) * PIMG].rearrange(
                        "p (r c) -> p r c", r=PW, c=PW)[
                        :, 1 + half * 14:1 + (half + 1) * 14, 1:PW]
                    src = pmm[:].rearrange("p (r c) -> p r c", r=14, c=28)
                    slot = m * 16 + img * 2 + half
                    sacc = p1s_c[:, slot:slot + 1]
                    qacc = p1q_c[:, slot:slot + 1]
                    if (img + half) % 2 == 0:
                        nc.vector.tensor_scalar(dst, src, 1.0, 0.0,
                                                OP.mult, OP.add,
                                                accum_out=sacc)
                        nc.scalar.activation(dum[:, 0:NCH], pmm[:],
                                             ACT.Square, accum_out=qacc)
                    else:
                        nc.scalar.activation(dst, src, ACT.Copy,
                                             accum_out=sacc)
                        nc.vector.scalar_tensor_tensor(
                            dumd[:, 0:NCH].rearrange(
                                "p (r c) -> p r c", r=14, c=28),
                            dst, 1.0, dst, OP.mult, OP.mult,
                            accum_out=qacc)

        # p1 stat reduction -> st2 -> AR2
        p1sv = p1s_c[:].rearrange("p (m c) -> p m c", c=16)
        p1qv = p1q_c[:].rearrange("p (m c) -> p m c", c=16)
        for m in range(KM1):
            nc.vector.reduce_sum(st2[:, m:m + 1], p1sv[:, m], axis=AX.X)
            nc.vector.reduce_sum(st2[:, KM1 + m:KM1 + m + 1], p1qv[:, m],
                                 axis=AX.X)
        ar2_in = dram.tile([128, 20], F32, name="ar2_in")
        ar2_out = dram.tile([128, 20], F32, name="ar2_out")
        nc.sync.dma_start(ar2_in[:], st2[:])
        nc.gpsimd.collective_compute(
            "AllReduce", OP.add, replica_groups=[list(range(NCORES))],
            ins=[ar2_in.opt()], outs=[ar2_out.opt()])
        nc.sync.dma_start(g2[:], ar2_out[:])

        # ---- post-AR2 column math ----
        mean_p1 = sb.tile([128, KM1], F32, name="mean_p1")
        var_p1 = sb.tile([128, KM1], F32, name="var_p1")
        a2f = sb.tile([128, KM1], F32, name="a2f")
        b2f = sb.tile([128, KM1], F32, name="b2f")
        v2 = sb.tile([128, KM1], F32, name="v2")
        b2fb = sb.tile([128, KM1], BF16, name="b2fb")
        tmp_m1 = sb.tile([128, KM1], F32, name="tmp_m1")
        nc.vector.tensor_scalar(mean_p1[:], g2[:, 0:2], 1.0 / NG1, None, OP.mult)
        nc.vector.tensor_scalar(var_p1[:], g2[:, 2:4], 1.0 / NG1, None, OP.mult)
        nc.vector.tensor_tensor(tmp_m1[:], mean_p1[:], mean_p1[:], OP.mult)
        nc.vector.tensor_tensor(var_p1[:], var_p1[:], tmp_m1[:], OP.subtract)
        nc.vector.tensor_tensor(tmp_m1[:], a1c, a1c, OP.mult)
        nc.vector.tensor_tensor(var_p1[:], var_p1[:], tmp_m1[:], OP.mult)
        rsqrt_cols(a2f[:], var_p1[:], bn2g, extra_mul=a1c)
        nc.vector.tensor_tensor(tmp_m1[:], a2f[:], mean_p1[:], OP.mult)
        nc.vector.tensor_tensor(b2f[:], bn2b, tmp_m1[:], OP.subtract)
        nc.vector.tensor_copy(b2fb[:], b2f[:])
        # v2 = -b2f/a2f = mean_p1 - bn2b/a2f
        nc.vector.reciprocal(tmp_m1[:], a2f[:])
        nc.vector.tensor_tensor(tmp_m1[:], bn2b, tmp_m1[:], OP.mult)
        nc.vector.tensor_tensor(v2[:], mean_p1[:], tmp_m1[:], OP.subtract)

        # ds BN affine (uses AR2 q stats)
        mean_q = sb.tile([128, KM2], F32, name="mean_q")
        var_q = sb.tile([128, KM2], F32, name="var_q")
        aq = sb.tile([128, KM2], F32, name="aq")
        bq = sb.tile([128, KM2], F32, name="bq")
        tmp_m2 = sb.tile([128, KM2], F32, name="tmp_m2")
        nc.vector.tensor_scalar(mean_q[:], g2[:, 4:12], 1.0 / NG2, None, OP.mult)
        nc.vector.tensor_scalar(var_q[:], g2[:, 12:20], 1.0 / NG2, None, OP.mult)
        nc.vector.tensor_tensor(tmp_m2[:], mean_q[:], mean_q[:], OP.mult)
        nc.vector.tensor_tensor(var_q[:], var_q[:], tmp_m2[:], OP.subtract)
        rsqrt_cols(aq[:], var_q[:], dsg)
        nc.vector.tensor_tensor(tmp_m2[:], aq[:], mean_q[:], OP.mult)
        nc.vector.tensor_tensor(bq[:], dsb, tmp_m2[:], OP.subtract)

        # fold a2f into t2 weights; fill p1 pads with v2
        for k in range(KM1):
            nc.vector.tensor_scalar(
                t2s[k][:].rearrange("p t o -> p (t o)"),
                t2s[k][:].rearrange("p t o -> p (t o)"),
                a2f[:, k:k + 1], None, OP.mult)
        for m in range(KM1):
            pv = p1[m][:].rearrange("p (i r c) -> p i r c", i=NI, r=PW, c=PW)
            nc.vector.tensor_copy(
                pv[:, :, 0, :], v2[:, m:m + 1].broadcast_to([128, NI, PW]))
            nc.vector.tensor_copy(
                pv[:, :, 1:PW, 0], v2[:, m:m + 1].broadcast_to([128, NI, 28]))

        # T[o] = sum_i S2sum[i,o] * b2f[i]  (border-free BN2 shift)
        Tc = sb.tile([128, KM1], F32, name="Tc")
        for m in range(KM1):
            tps = ps_b.tile([128, NCH], F32, tag="c2", name=f"tps_{m}")
            for k in range(KM1):
                nc.tensor.matmul(tps[:, 0:1], s2s[k][:, m * 128:(m + 1) * 128],
                                 b2fb[:, k:k + 1],
                                 start=(k == 0), stop=(k == KM1 - 1))
            nc.vector.tensor_copy(Tc[:, m:m + 1], tps[:, 0:1])

        # ================= conv2: 3x3 s2 p1 from padded p1 =================
        p1v = [p1[k][:].rearrange("p (i r c) -> p i r c", i=NI, r=PW, c=PW)
               for k in range(KM1)]
        for m in range(KM1):
            pmms = [ps_b.tile([128, NCH], F32, tag="c2",
                              name=f"psc2_{m}_{ip}") for ip in range(4)]
            for t, (dy, dx) in enumerate(TAPS):
                for k in range(KM1):
                    for ip in range(4):
                        rhs = p1v[k][:, 2 * ip:2 * ip + 2,
                                     dy:dy + 27:2, dx:dx + 27:2]
                        nc.tensor.matmul(
                            pmms[ip][:],
                            t2s[k][:, t, m * 128:(m + 1) * 128],
                            rhs, start=(t == 0 and k == 0),
                            stop=(t == 8 and k == KM1 - 1))
            for ip in range(4):
                slot = m * 4 + ip
                nc.vector.tensor_scalar(
                    p2[m][:, ip * NCH:(ip + 1) * NCH], pmms[ip][:],
                    1.0, Tc[:, m:m + 1], OP.mult, OP.add,
                    accum_out=p2s_c[:, slot:slot + 1])
                nc.scalar.activation(
                    dum[:, 0:NCH], p2[m][:, ip * NCH:(ip + 1) * NCH],
                    ACT.Square, accum_out=p2q_c[:, slot:slot + 1])

        # p2 stats -> st3 -> AR3
        for m in range(KM1):
            nc.vector.reduce_sum(st3[:, m:m + 1],
                                 p2s_c[:, m * 4:(m + 1) * 4], axis=AX.X)
            nc.vector.reduce_sum(st3[:, 2 + m:3 + m],
                                 p2q_c[:, m * 4:(m + 1) * 4], axis=AX.X)
        ar3_in = dram.tile([128, 4], F32, name="ar3_in")
        ar3_out = dram.tile([128, 4], F32, name="ar3_out")
        nc.sync.dma_start(ar3_in[:], st3[:])
        nc.gpsimd.collective_compute(
            "AllReduce", OP.add, replica_groups=[list(range(NCORES))],
            ins=[ar3_in.opt()], outs=[ar3_out.opt()])
        nc.sync.dma_start(g3[:], ar3_out[:])

        # shortcut affine in place on q — overlaps AR3 (needs only AR2 stats)
        for m in range(KM2):
            nc.vector.tensor_scalar(q[m][:], q[m][:], aq[:, m:m + 1],
                                    bq[:, m:m + 1], OP.mult, OP.add)

        # ---- BN3 affine (alpha2-corrected) -> xn3 in place on p2 ----
        mean_p2 = sb.tile([128, KM1], F32, name="mean_p2")
        var_p2 = sb.tile([128, KM1], F32, name="var_p2")
        a3f = sb.tile([128, KM1], F32, name="a3f")
        b3f = sb.tile([128, KM1], F32, name="b3f")
        tmp_m3 = sb.tile([128, KM1], F32, name="tmp_m3")
        nc.vector.tensor_scalar(mean_p2[:], g3[:, 0:2], 1.0 / NG2, None, OP.mult)
        nc.vector.tensor_scalar(var_p2[:], g3[:, 2:4], 1.0 / NG2, None, OP.mult)
        nc.vector.tensor_tensor(tmp_m3[:], mean_p2[:], mean_p2[:], OP.mult)
        nc.vector.tensor_tensor(var_p2[:], var_p2[:], tmp_m3[:], OP.subtract)
        nc.vector.tensor_tensor(tmp_m3[:], a2c, a2c, OP.mult)
        nc.vector.tensor_tensor(var_p2[:], var_p2[:], tmp_m3[:], OP.mult)
        rsqrt_cols(a3f[:], var_p2[:], bn3g, extra_mul=a2c)
        nc.vector.tensor_tensor(tmp_m3[:], a3f[:], mean_p2[:], OP.mult)
        nc.vector.tensor_tensor(b3f[:], bn3b, tmp_m3[:], OP.subtract)
        for m in range(KM1):
            nc.vector.tensor_scalar(p2[m][:], p2[m][:], a3f[:, m:m + 1],
                                    b3f[:, m:m + 1], OP.mult, OP.add)

        # ================= conv3 + residual fuse + store =================
        for m in range(KM2):
            for j in range(4):
                pmm = ps_a.tile([128, NCH], F32, tag="mm",
                                name=f"psc3_{m}_{j}")
                for k in range(KM1):
                    nc.tensor.matmul(
                        pmm[:], t3s[k][:, m * 128:(m + 1) * 128],
                        p2[k][:, j * NCH:(j + 1) * NCH],
                        start=(k == 0), stop=(k == KM1 - 1))
                out_t = sb_out.tile([128, NCH], BF16, tag="out",
                                    name=f"out_{m}_{j}")
                qc = q[m][:, j * NCH:(j + 1) * NCH]
                if (m + j) % 2 == 0:
                    nc.vector.scalar_tensor_tensor(
                        out_t[:], pmm[:], a3c[:, m:m + 1], qc,
                        OP.mult, OP.add)
                else:
                    # ACT scales out of psum, DVE adds the shortcut (2x)
                    nc.scalar.activation(out_t[:], pmm[:], ACT.Copy,
                                         scale=a3c[:, m:m + 1])
                    nc.vector.tensor_tensor(out_t[:], out_t[:], qc, OP.add)
                nc.sync.dma_start(
                    out_d.ap()[m * 128:(m + 1) * 128,
                               j * NCH:(j + 1) * NCH],
                    out_t[:])


def _prep_host(inputs):
    """Host-side: shard x, fold weights, build per-core in_maps."""
    x = np.asarray(inputs["x"], np.float32)          # [64, 512, 28, 28]
    w1 = np.asarray(inputs["w1"], np.float32)
    w2 = np.asarray(inputs["w2"], np.float32)
    w3 = np.asarray(inputs["w3"], np.float32)
    ds_w = np.asarray(inputs["ds_w"], np.float32)

    s1, al1 = _ternarize_host(w1)    # [256,512,1,1]
    s2, al2 = _ternarize_host(w2)    # [256,256,3,3]
    s3, al3 = _ternarize_host(w3)    # [1024,256,1,1]

    t1 = np.ascontiguousarray(s1[:, :, 0, 0].T).astype(BF)       # [512, 256]
    # t2[t, i, o] = s2[o, i, ky, kx], t = ky*3+kx; shipped pre-tiled as
    # [k, i_local, t*C1 + o] so each partition row is one contiguous DMA
    t2 = s2.transpose(2, 3, 1, 0).reshape(9, C1, C1)
    s2sum = t2.sum(axis=0).astype(BF)                            # [256, 256]
    t2k = np.ascontiguousarray(
        t2.transpose(1, 0, 2).reshape(KM1, 128, 9 * C1)).astype(BF)
    t3 = np.ascontiguousarray(s3[:, :, 0, 0].T).astype(BF)       # [256, 1024]
    dsw = np.ascontiguousarray(ds_w[:, :, 0, 0].T).astype(BF)    # [512, 1024]

    def cols(v, n):
        # [n*128] channel vector -> [128, n] column layout
        return np.asarray(v, np.float32).reshape(n, 128).T

    par = np.concatenate([
        cols(al1, 2), cols(al2, 2), cols(al3, 8),
        cols(inputs["bn1_g"], 4), cols(inputs["bn2_g"], 2),
        cols(inputs["bn2_b"], 2), cols(inputs["bn3_g"], 2),
        cols(inputs["bn3_b"], 2), cols(inputs["ds_bn_g"], 8),
        cols(inputs["ds_bn_b"], 8),
    ], axis=1)
    par = np.ascontiguousarray(par, dtype=np.float32)            # [128, 40]

    common = dict(t1=t1, t2=t2k, s2sum=s2sum, t3=t3, dsw=dsw, par=par)

    in_maps = []
    for c in range(NCORES):
        xc = x[c * NI:(c + 1) * NI]                      # [8, 512, 28, 28]
        xc = np.ascontiguousarray(
            xc.transpose(1, 0, 2, 3).reshape(C0, PIX1)).astype(BF)
        in_maps.append({"x": xc, **common})
    return in_maps


def kernel(**inputs):
    if "nc" not in _CACHE:
        _CACHE["nc"] = build_program()
    nc = _CACHE["nc"]

    in_maps = _prep_host(inputs)
    try:
        res = run_bass_kernel_spmd(nc, in_maps, core_ids=list(range(NCORES)))
    except Exception:
        # transient device state (e.g. a previous crashed run) usually
        # clears on retry
        res = run_bass_kernel_spmd(nc, in_maps, core_ids=list(range(NCORES)))

    out = np.empty((64, C2, H2, H2), np.float32)
    for c in range(NCORES):
        oc = np.asarray(res.results[c]["out"]).astype(np.float32)
        oc = oc.reshape(C2, NI, H2, H2)
        out[c * NI:(c + 1) * NI] = oc.transpose(1, 0, 2, 3)
    return out


# revision 39
# speedup vs baseline: 1.3520x; 1.0867x over previous
"""Trainium2 Bass kernel for nn_BottleneckR (pre-activation ternary bottleneck).

Reference (batch 64):
  h  = conv1x1(BN1(x), tern(w1))            [64,256,28,28]
  h  = conv3x3s2p1(BN2(h), tern(w2))        [64,256,14,14]
  h  = conv1x1(BN3(h), tern(w3))            [64,1024,14,14]
  sc = BN_ds(conv1x1s2(x, ds_w))            [64,1024,14,14]
  out = h + sc

Strategy: data-parallel over batch on 8 NeuronCores (8 images/core).
Exact sync-BN via 3 small AllReduces of per-channel sum/sumsq.

Differences vs the v1 kernel (306us):
 - bf16 end to end: x is converted to bf16 on the host (halves the HBM
   stream), all matmuls run bf16 at full PE rate, DVE elementwise ops get
   the 2x 16-bit mode. Ternary sign matrices are exact in bf16.
 - x stays resident in SBUF (one HBM read instead of two).
 - p1 is stored in a 29x29 zero-padded per-image layout; conv2's 9 taps
   read it directly with stride-2 access patterns (no phase-split scatter,
   no shifted-plane copies).
 - BN2 is folded into conv2 instead of materialized: the scale a2f folds
   into the t2 weights (per input channel), the shift b2f is handled by
   adding T[o] = sum_i S2sum[i,o]*b2f[i] at the psum->p2 copy while the
   pad cells hold -b2f/a2f (so border taps contribute zero shift, exactly
   matching zero padding after BN).
 - AR1 is triggered as soon as the x statistics are done; the ds-conv
   tail, q sum-of-squares and all q/p1 copies overlap the collective.
 - q's BN affine runs during AR3's latency window.
 - One persistent tile pool: no mid-kernel pool-exit barriers (the v1
   kernel lost ~16us of DVE time to one of those after AR3).
"""

import sys

sys.path.insert(0, "/opt/trn_rl_repo")

import numpy as np
import ml_dtypes

import concourse.bacc as bacc
import concourse.mybir as mybir
import concourse.tile as tile
from concourse.bass_utils import run_bass_kernel_spmd

F32 = mybir.dt.float32
BF16 = mybir.dt.bfloat16
AX = mybir.AxisListType
OP = mybir.AluOpType
ACT = mybir.ActivationFunctionType

NCORES = 8
NI = 8                                   # images per core
C0, C1, C2 = 512, 256, 1024
H1, HW1 = 28, 784
H2, HW2 = 14, 196
PIX1, PIX2 = NI * HW1, NI * HW2          # 6272, 1568
K0, KM1, KM2 = C0 // 128, C1 // 128, C2 // 128   # 4, 2, 8
NCH = 392                                # pixel chunk (2 imgs at 14x14 / half img at 28x28)
PW = 29                                  # padded image width (top/left zero pad)
PIMG = PW * PW                           # 841
NG1 = 64 * HW1
NG2 = 64 * HW2
EPS = 1e-5
BF = ml_dtypes.bfloat16

TAPS = [(dy, dx) for dy in range(3) for dx in range(3)]

_CACHE = {}


def _ternarize_host(w):
    """fp32 ternarize matching the jax reference: returns (S, alpha)."""
    w = np.asarray(w, np.float32)
    absw = np.abs(w)
    delta = (0.7 * absw.mean(axis=(1, 2, 3), keepdims=True)).astype(np.float32)
    mask = (absw > delta).astype(np.float32)
    alpha = (absw * mask).sum(axis=(1, 2, 3)) / (mask.sum(axis=(1, 2, 3)) + 1e-8)
    sign = np.sign(w).astype(np.float32)
    return (sign * mask).astype(np.float32), alpha.astype(np.float32)


def build_program():
    nc = bacc.Bacc("TRN2", target_bir_lowering=False, debug=False,
                   num_devices=NCORES)

    x_d = nc.dram_tensor("x", [C0, PIX1], BF16, kind="ExternalInput")
    t1_d = nc.dram_tensor("t1", [C0, C1], BF16, kind="ExternalInput")
    # t2 pre-arranged on host: [k, i_local, t*C1+o]
    t2_d = nc.dram_tensor("t2", [KM1, 128, 9 * C1], BF16, kind="ExternalInput")
    s2s_d = nc.dram_tensor("s2sum", [C1, C1], BF16, kind="ExternalInput")
    t3_d = nc.dram_tensor("t3", [C1, C2], BF16, kind="ExternalInput")
    dsw_d = nc.dram_tensor("dsw", [C0, C2], BF16, kind="ExternalInput")
    # all per-channel params pre-arranged on host as [128, n] column tiles
    # (one contiguous DMA row per partition instead of thousands of 4B
    # descriptors): [a1(2) a2(2) a3(8) bn1g(4) bn2g(2) bn2b(2) bn3g(2)
    # bn3b(2) dsg(8) dsb(8)] = 40 cols
    par_d = nc.dram_tensor("par", [128, 40], F32, kind="ExternalInput")
    out_d = nc.dram_tensor("out", [C2, PIX2], BF16, kind="ExternalOutput")

    with tile.TileContext(nc) as tc:
        _build(nc, tc, x_d, t1_d, t2_d, s2s_d, t3_d, dsw_d, par_d, out_d)

    nc.compile()
    return nc


def _build(nc, tc, x_d, t1_d, t2_d, s2s_d, t3_d, dsw_d, par_d, out_d):
    from contextlib import ExitStack

    gctx = ExitStack()
    with gctx:
        dram = gctx.enter_context(tc.tile_pool(name="dram", bufs=1, space="DRAM"))
        sb = gctx.enter_context(tc.tile_pool(name="sb", bufs=1))
        sb_out = gctx.enter_context(tc.tile_pool(name="sb_out", bufs=4))
        ps_a = gctx.enter_context(tc.tile_pool(name="ps_a", bufs=4, space="PSUM"))
        ps_b = gctx.enter_context(tc.tile_pool(name="ps_b", bufs=4, space="PSUM"))

        # ------------- static loads -------------
        # tiles declared first; DMAs issued in priority order below
        t1s = [sb.tile([128, C1], BF16, name=f"t1_{k}") for k in range(K0)]
        dsws = [sb.tile([128, C2], BF16, name=f"dsw_{k}") for k in range(K0)]
        t2s = [sb.tile([128, 9, C1], BF16, name=f"t2_{k}")
               for k in range(KM1)]
        s2s = [sb.tile([128, C1], BF16, name=f"s2s_{k}") for k in range(KM1)]
        t3s = [sb.tile([128, C2], BF16, name=f"t3_{k}") for k in range(KM1)]
        par = sb.tile([128, 40], F32, name="par")
        a1c = par[:, 0:2]
        a2c = par[:, 2:4]
        a3c = par[:, 4:12]
        bn1g = par[:, 12:16]
        bn2g = par[:, 16:18]
        bn2b = par[:, 18:20]
        bn3g = par[:, 20:22]
        bn3b = par[:, 22:24]
        dsg = par[:, 24:32]
        dsb = par[:, 32:40]

        # persistent activations
        xs = [sb.tile([128, PIX1], BF16, name=f"xs_{k}") for k in range(K0)]
        p1 = [sb.tile([128, NI * PIMG], BF16, name=f"p1_{m}")
              for m in range(KM1)]
        q = [sb.tile([128, PIX2], BF16, name=f"q_{m}") for m in range(KM2)]
        p2 = [sb.tile([128, PIX2], BF16, name=f"p2_{m}") for m in range(KM1)]
        dum = sb.tile([128, PIX2], BF16, name="dum")     # ACT scratch
        dum2 = sb.tile([128, PIX2], BF16, name="dum2")   # Pool scratch
        dumd = sb.tile([128, PIX2], BF16, name="dumd")   # DVE scratch

        # stat tiles
        xbn = sb.tile([128, 40 * 6], F32, name="xbn")
        xagg = sb.tile([128, 3 * 2], F32, name="xagg")
        xsum_c = sb.tile([128, K0 * 4], F32, name="xsum_c")
        xsq_c = sb.tile([128, K0 * 4], F32, name="xsq_c")
        st1 = sb.tile([128, 2 * K0], F32, name="st1")
        g1 = sb.tile([128, 2 * K0], F32, name="g1")
        qsum_c = sb.tile([128, KM2 * 4], F32, name="qsum_c")
        p1s_c = sb.tile([128, KM1 * 16], F32, name="p1s_c")
        p1q_c = sb.tile([128, KM1 * 16], F32, name="p1q_c")
        st2 = sb.tile([128, 20], F32, name="st2")
        g2 = sb.tile([128, 20], F32, name="g2")
        p2s_c = sb.tile([128, KM1 * 4], F32, name="p2s_c")
        p2q_c = sb.tile([128, KM1 * 4], F32, name="p2q_c")
        st3 = sb.tile([128, 4], F32, name="st3")
        g3 = sb.tile([128, 4], F32, name="g3")

        # DMA issue order: x pair 0 first (unblocks stats + ds conv), then
        # the weights the first ops need, then the remaining x pairs, then
        # everything not needed until later phases.
        def load_pair(p):
            for k in range(K0):
                nc.sync.dma_start(
                    xs[k][:, p * 2 * HW1:(p + 1) * 2 * HW1],
                    x_d.ap()[k * 128:(k + 1) * 128,
                             p * 2 * HW1:(p + 1) * 2 * HW1])

        load_pair(0)
        for k in range(K0):
            nc.sync.dma_start(dsws[k][:],
                              dsw_d.ap()[k * 128:(k + 1) * 128, :])
        for p in range(1, 4):
            load_pair(p)
        for k in range(K0):
            nc.sync.dma_start(t1s[k][:], t1_d.ap()[k * 128:(k + 1) * 128, :])
        nc.sync.dma_start(par[:], par_d.ap())
        for k in range(KM1):
            nc.sync.dma_start(
                t2s[k][:].rearrange("p t o -> p (t o)"), t2_d.ap()[k])
            nc.sync.dma_start(s2s[k][:],
                              s2s_d.ap()[k * 128:(k + 1) * 128, :])
            nc.sync.dma_start(t3s[k][:],
                              t3_d.ap()[k * 128:(k + 1) * 128, :])

        # ================= phase A: x stats + ds conv =================
        # x statistics: bn_stats (sum+var in one 1.19ns/elem pass) on DVE
        # for 10 chunks; ACT takes the remaining 6 as Copy-accum (sum) +
        # Square-accum (sumsq). Accumulator ops get no 16-bit speedup, so
        # this split balances the lanes.
        NBN = {0: 4, 1: 4, 2: 2, 3: 0}    # bn_stats chunks per k (pairs 0..)
        for p in range(4):
            for k in range(K0):
                chunk = xs[k][:, p * 1568:(p + 1) * 1568]
                c = k * 4 + p
                if p < NBN[k]:
                    base = c * 4
                    for g in range(4):
                        nc.vector.bn_stats(
                            xbn[:, (base + g) * 6:(base + g + 1) * 6],
                            xs[k][:, p * 1568 + g * 392:
                                  p * 1568 + (g + 1) * 392])
                else:
                    nc.scalar.activation(
                        dum[:], chunk, ACT.Copy,
                        accum_out=xsum_c[:, c:c + 1])
                    nc.scalar.activation(
                        dum[:], chunk, ACT.Square,
                        accum_out=xsq_c[:, c:c + 1])
            for m in range(KM2):
                pool = ps_a if m % 2 == 0 else ps_b
                tagn = "mm" if m % 2 == 0 else "c2"
                pmm = pool.tile([128, NCH], F32, tag=tagn,
                                name=f"psds_{p}_{m}")
                for k in range(K0):
                    rhs = xs[k][:, p * 1568:(p + 1) * 1568].rearrange(
                        "p (i ay by ax bx) -> p i by bx ay ax",
                        i=2, ay=14, by=2, ax=14, bx=2)[:, :, 0, 0, :, :]
                    nc.tensor.matmul(
                        pmm[:], dsws[k][:, m * 128:(m + 1) * 128],
                        rhs, start=(k == 0), stop=(k == K0 - 1))
                dst = q[m][:, p * NCH:(p + 1) * NCH]
                acc = qsum_c[:, m * 4 + p:m * 4 + p + 1]
                if m % 2 == 0:
                    nc.vector.tensor_scalar(dst, pmm[:], 1.0, 0.0,
                                            OP.mult, OP.add, accum_out=acc)
                else:
                    nc.scalar.activation(dst, pmm[:], ACT.Copy, accum_out=acc)

        # x stat reduction -> st1 -> AR1
        # bn side: per k aggregate its bn_stats groups into sum / sumsq
        xbnv = xbn[:].rearrange("p (g s) -> p g s", s=6)
        tmp1 = sb.tile([128, 1], F32, tag="xa", name="xa_tmp1")
        tmp2 = sb.tile([128, 1], F32, tag="xa", name="xa_tmp2")
        for k in range(3):
            ng = NBN[k] * 4
            g0 = (k * 16)
            nc.vector.bn_aggr(xagg[:, k * 2:k * 2 + 2],
                              xbnv[:, g0:g0 + ng, :])
            cnt = float(NBN[k] * 1568)
            mcol = xagg[:, k * 2:k * 2 + 1]
            vcol = xagg[:, k * 2 + 1:k * 2 + 2]
            nc.vector.tensor_scalar(st1[:, k:k + 1], mcol, cnt, None, OP.mult)
            nc.vector.tensor_tensor(tmp1[:], mcol, mcol, OP.mult)
            nc.vector.tensor_tensor(tmp1[:], tmp1[:], vcol, OP.add)
            nc.vector.tensor_scalar(st1[:, K0 + k:K0 + k + 1], tmp1[:],
                                    cnt, None, OP.mult)
        # ACT side: add the accumulated sums for chunks not covered by bn
        nc.vector.tensor_tensor(tmp1[:], xsum_c[:, 10:11], xsum_c[:, 11:12],
                                OP.add)
        nc.vector.tensor_tensor(st1[:, 2:3], st1[:, 2:3], tmp1[:], OP.add)
        nc.vector.tensor_tensor(tmp2[:], xsq_c[:, 10:11], xsq_c[:, 11:12],
                                OP.add)
        nc.vector.tensor_tensor(st1[:, K0 + 2:K0 + 3], st1[:, K0 + 2:K0 + 3],
                                tmp2[:], OP.add)
        nc.vector.reduce_sum(st1[:, 3:4], xsum_c[:, 12:16], axis=AX.X)
        nc.vector.reduce_sum(st1[:, K0 + 3:K0 + 4], xsq_c[:, 12:16],
                             axis=AX.X)
        ar1_in = dram.tile([128, 2 * K0], F32, name="ar1_in")
        ar1_out = dram.tile([128, 2 * K0], F32, name="ar1_out")
        nc.sync.dma_start(ar1_in[:], st1[:])
        nc.gpsimd.collective_compute(
            "AllReduce", OP.add, replica_groups=[list(range(NCORES))],
            ins=[ar1_in.opt()], outs=[ar1_out.opt()])
        nc.sync.dma_start(g1[:], ar1_out[:])

        # q sumsq (ACT) + q sum reduce (DVE) — overlap AR1
        for m in range(KM2):
            nc.scalar.activation(dum[:], q[m][:], ACT.Square,
                                 accum_out=st2[:, 12 + m:13 + m])
            nc.vector.reduce_sum(st2[:, 4 + m:5 + m],
                                 qsum_c[:, m * 4:(m + 1) * 4], axis=AX.X)

        # ---- BN1 affine -> fold into t1 ----
        mean_x = sb.tile([128, K0], F32, name="mean_x")
        var_x = sb.tile([128, K0], F32, name="var_x")
        a1f = sb.tile([128, K0], F32, name="a1f")
        tmp_k0 = sb.tile([128, K0], F32, name="tmp_k0")
        nc.vector.tensor_scalar(mean_x[:], g1[:, 0:K0], 1.0 / NG1, None, OP.mult)
        nc.vector.tensor_tensor(tmp_k0[:], mean_x[:], mean_x[:], OP.mult)
        nc.vector.tensor_scalar(var_x[:], g1[:, K0:2 * K0], 1.0 / NG1, None,
                                OP.mult)
        nc.vector.tensor_tensor(var_x[:], var_x[:], tmp_k0[:], OP.subtract)

        def rsqrt_cols(dst, var_ap, gamma_ap, extra_mul=None):
            cols = dst.shape[1]
            tmp = sb.tile([128, cols], F32, tag="rsq_tmp",
                          name=f"rsq_{dst.tensor.name}")
            nc.vector.tensor_scalar(tmp[:], var_ap, EPS, None, OP.add)
            nc.vector.reciprocal(tmp[:], tmp[:])
            nc.scalar.sqrt(tmp[:], tmp[:])
            nc.vector.tensor_tensor(dst, tmp[:], gamma_ap, OP.mult)
            if extra_mul is not None:
                nc.vector.tensor_tensor(dst, dst, extra_mul, OP.mult)

        rsqrt_cols(a1f[:], var_x[:], bn1g)
        for k in range(K0):
            nc.vector.tensor_scalar(t1s[k][:], t1s[k][:], a1f[:, k:k + 1],
                                    None, OP.mult)

        # ================= conv1 (raw x @ folded t1) =================
        # psum->p1 copies alternate DVE/ACT (both with sum accumulators);
        # sumsq per chunk on Pool (scalar_tensor_tensor square).
        for m in range(KM1):
            for img in range(NI):
                for half in range(2):
                    pmm = ps_a.tile([128, NCH], F32, tag="mm",
                                    name=f"psc1_{m}_{img}_{half}")
                    for k in range(K0):
                        nc.tensor.matmul(
                            pmm[:], t1s[k][:, m * 128:(m + 1) * 128],
                            xs[k][:, img * HW1 + half * NCH:
                                  img * HW1 + (half + 1) * NCH],
                            start=(k == 0), stop=(k == K0 - 1))
                    dst = p1[m][:, img * PIMG:(img + 1) * PIMG].rearrange(
                        "p (r c) -> p r c", r=PW, c=PW)[
                        :, 1 + half * 14:1 + (half + 1) * 14, 1:PW]
                    src = pmm[:].rearrange("p (r c) -> p r c", r=14, c=28)
                    slot = m * 16 + img * 2 + half
                    sacc = p1s_c[:, slot:slot + 1]
                    qacc = p1q_c[:, slot:slot + 1]
                    if (img + half) % 2 == 0:
                        nc.vector.tensor_scalar(dst, src, 1.0, 0.0,
                                                OP.mult, OP.add,
                                                accum_out=sacc)
                        nc.scalar.activation(dum[:, 0:NCH], pmm[:],
                                             ACT.Square, accum_out=qacc)
                    else:
                        nc.scalar.activation(dst, src, ACT.Copy,
                                             accum_out=sacc)
                        nc.vector.scalar_tensor_tensor(
                            dumd[:, 0:NCH].rearrange(
                                "p (r c) -> p r c", r=14, c=28),
                            dst, 1.0, dst, OP.mult, OP.mult,
                            accum_out=qacc)

        # p1 stat reduction -> st2 -> AR2
        p1sv = p1s_c[:].rearrange("p (m c) -> p m c", c=16)
        p1qv = p1q_c[:].rearrange("p (m c) -> p m c", c=16)
        for m in range(KM1):
            nc.vector.reduce_sum(st2[:, m:m + 1], p1sv[:, m], axis=AX.X)
            nc.vector.reduce_sum(st2[:, KM1 + m:KM1 + m + 1], p1qv[:, m],
                                 axis=AX.X)
        ar2_in = dram.tile([128, 20], F32, name="ar2_in")
        ar2_out = dram.tile([128, 20], F32, name="ar2_out")
        nc.sync.dma_start(ar2_in[:], st2[:])
        nc.gpsimd.collective_compute(
            "AllReduce", OP.add, replica_groups=[list(range(NCORES))],
            ins=[ar2_in.opt()], outs=[ar2_out.opt()])
        nc.sync.dma_start(g2[:], ar2_out[:])

        # ---- post-AR2 column math ----
        mean_p1 = sb.tile([128, KM1], F32, name="mean_p1")
        var_p1 = sb.tile([128, KM1], F32, name="var_p1")
        a2f = sb.tile([128, KM1], F32, name="a2f")
        b2f = sb.tile([128, KM1], F32, name="b2f")
        v2 = sb.tile([128, KM1], F32, name="v2")
        b2fb = sb.tile([128, KM1], BF16, name="b2fb")
        tmp_m1 = sb.tile([128, KM1], F32, name="tmp_m1")
        nc.vector.tensor_scalar(mean_p1[:], g2[:, 0:2], 1.0 / NG1, None, OP.mult)
        nc.vector.tensor_scalar(var_p1[:], g2[:, 2:4], 1.0 / NG1, None, OP.mult)
        nc.vector.tensor_tensor(tmp_m1[:], mean_p1[:], mean_p1[:], OP.mult)
        nc.vector.tensor_tensor(var_p1[:], var_p1[:], tmp_m1[:], OP.subtract)
        nc.vector.tensor_tensor(tmp_m1[:], a1c, a1c, OP.mult)
        nc.vector.tensor_tensor(var_p1[:], var_p1[:], tmp_m1[:], OP.mult)
        rsqrt_cols(a2f[:], var_p1[:], bn2g, extra_mul=a1c)
        nc.vector.tensor_tensor(tmp_m1[:], a2f[:], mean_p1[:], OP.mult)
        nc.vector.tensor_tensor(b2f[:], bn2b, tmp_m1[:], OP.subtract)
        nc.vector.tensor_copy(b2fb[:], b2f[:])
        # v2 = -b2f/a2f = mean_p1 - bn2b/a2f
        nc.vector.reciprocal(tmp_m1[:], a2f[:])
        nc.vector.tensor_tensor(tmp_m1[:], bn2b, tmp_m1[:], OP.mult)
        nc.vector.tensor_tensor(v2[:], mean_p1[:], tmp_m1[:], OP.subtract)

        # ds BN affine (uses AR2 q stats)
        mean_q = sb.tile([128, KM2], F32, name="mean_q")
        var_q = sb.tile([128, KM2], F32, name="var_q")
        aq = sb.tile([128, KM2], F32, name="aq")
        bq = sb.tile([128, KM2], F32, name="bq")
        tmp_m2 = sb.tile([128, KM2], F32, name="tmp_m2")
        nc.vector.tensor_scalar(mean_q[:], g2[:, 4:12], 1.0 / NG2, None, OP.mult)
        nc.vector.tensor_scalar(var_q[:], g2[:, 12:20], 1.0 / NG2, None, OP.mult)
        nc.vector.tensor_tensor(tmp_m2[:], mean_q[:], mean_q[:], OP.mult)
        nc.vector.tensor_tensor(var_q[:], var_q[:], tmp_m2[:], OP.subtract)
        rsqrt_cols(aq[:], var_q[:], dsg)
        nc.vector.tensor_tensor(tmp_m2[:], aq[:], mean_q[:], OP.mult)
        nc.vector.tensor_tensor(bq[:], dsb, tmp_m2[:], OP.subtract)

        # fold a2f into t2 weights; fill p1 pads with v2
        for k in range(KM1):
            nc.vector.tensor_scalar(
                t2s[k][:].rearrange("p t o -> p (t o)"),
                t2s[k][:].rearrange("p t o -> p (t o)"),
                a2f[:, k:k + 1], None, OP.mult)
        for m in range(KM1):
            pv = p1[m][:].rearrange("p (i r c) -> p i r c", i=NI, r=PW, c=PW)
            nc.vector.tensor_copy(
                pv[:, :, 0, :], v2[:, m:m + 1].broadcast_to([128, NI, PW]))
            nc.vector.tensor_copy(
                pv[:, :, 1:PW, 0], v2[:, m:m + 1].broadcast_to([128, NI, 28]))

        # T[o] = sum_i S2sum[i,o] * b2f[i]  (border-free BN2 shift)
        Tc = sb.tile([128, KM1], F32, name="Tc")
        for m in range(KM1):
            tps = ps_b.tile([128, NCH], F32, tag="c2", name=f"tps_{m}")
            for k in range(KM1):
                nc.tensor.matmul(tps[:, 0:1], s2s[k][:, m * 128:(m + 1) * 128],
                                 b2fb[:, k:k + 1],
                                 start=(k == 0), stop=(k == KM1 - 1))
            nc.vector.tensor_copy(Tc[:, m:m + 1], tps[:, 0:1])

        # ================= conv2: 3x3 s2 p1 from padded p1 =================
        p1v = [p1[k][:].rearrange("p (i r c) -> p i r c", i=NI, r=PW, c=PW)
               for k in range(KM1)]
        for m in range(KM1):
            pmms = [ps_b.tile([128, NCH], F32, tag="c2",
                              name=f"psc2_{m}_{ip}") for ip in range(4)]
            for t, (dy, dx) in enumerate(TAPS):
                for k in range(KM1):
                    for ip in range(4):
                        rhs = p1v[k][:, 2 * ip:2 * ip + 2,
                                     dy:dy + 27:2, dx:dx + 27:2]
                        nc.tensor.matmul(
                            pmms[ip][:],
                            t2s[k][:, t, m * 128:(m + 1) * 128],
                            rhs, start=(t == 0 and k == 0),
                            stop=(t == 8 and k == KM1 - 1))
            for ip in range(4):
                slot = m * 4 + ip
                nc.vector.tensor_scalar(
                    p2[m][:, ip * NCH:(ip + 1) * NCH], pmms[ip][:],
                    1.0, Tc[:, m:m + 1], OP.mult, OP.add,
                    accum_out=p2s_c[:, slot:slot + 1])
                nc.scalar.activation(
                    dum[:, 0:NCH], p2[m][:, ip * NCH:(ip + 1) * NCH],
                    ACT.Square, accum_out=p2q_c[:, slot:slot + 1])

        # p2 stats -> st3 -> AR3
        for m in range(KM1):
            nc.vector.reduce_sum(st3[:, m:m + 1],
                                 p2s_c[:, m * 4:(m + 1) * 4], axis=AX.X)
            nc.vector.reduce_sum(st3[:, 2 + m:3 + m],
                                 p2q_c[:, m * 4:(m + 1) * 4], axis=AX.X)
        ar3_in = dram.tile([128, 4], F32, name="ar3_in")
        ar3_out = dram.tile([128, 4], F32, name="ar3_out")
        nc.sync.dma_start(ar3_in[:], st3[:])
        nc.gpsimd.collective_compute(
            "AllReduce", OP.add, replica_groups=[list(range(NCORES))],
            ins=[ar3_in.opt()], outs=[ar3_out.opt()])
        nc.sync.dma_start(g3[:], ar3_out[:])

        # shortcut affine in place on q — overlaps AR3 (needs only AR2 stats)
        for m in range(KM2):
            nc.vector.tensor_scalar(q[m][:], q[m][:], aq[:, m:m + 1],
                                    bq[:, m:m + 1], OP.mult, OP.add)

        # ---- BN3 affine (alpha2-corrected) -> xn3 in place on p2 ----
        mean_p2 = sb.tile([128, KM1], F32, name="mean_p2")
        var_p2 = sb.tile([128, KM1], F32, name="var_p2")
        a3f = sb.tile([128, KM1], F32, name="a3f")
        b3f = sb.tile([128, KM1], F32, name="b3f")
        tmp_m3 = sb.tile([128, KM1], F32, name="tmp_m3")
        nc.vector.tensor_scalar(mean_p2[:], g3[:, 0:2], 1.0 / NG2, None, OP.mult)
        nc.vector.tensor_scalar(var_p2[:], g3[:, 2:4], 1.0 / NG2, None, OP.mult)
        nc.vector.tensor_tensor(tmp_m3[:], mean_p2[:], mean_p2[:], OP.mult)
        nc.vector.tensor_tensor(var_p2[:], var_p2[:], tmp_m3[:], OP.subtract)
        nc.vector.tensor_tensor(tmp_m3[:], a2c, a2c, OP.mult)
        nc.vector.tensor_tensor(var_p2[:], var_p2[:], tmp_m3[:], OP.mult)
        rsqrt_cols(a3f[:], var_p2[:], bn3g, extra_mul=a2c)
        nc.vector.tensor_tensor(tmp_m3[:], a3f[:], mean_p2[:], OP.mult)
        nc.vector.tensor_tensor(b3f[:], bn3b, tmp_m3[:], OP.subtract)
        for m in range(KM1):
            nc.vector.tensor_scalar(p2[m][:], p2[m][:], a3f[:, m:m + 1],
                                    b3f[:, m:m + 1], OP.mult, OP.add)

        # ================= conv3 + residual fuse + store =================
        for m in range(KM2):
            for j in range(4):
                pmm = ps_a.tile([128, NCH], F32, tag="mm",
                                name=f"psc3_{m}_{j}")
                for k in range(KM1):
                    nc.tensor.matmul(
                        pmm[:], t3s[k][:, m * 128:(m + 1) * 128],
                        p2[k][:, j * NCH:(j + 1) * NCH],
                        start=(k == 0), stop=(k == KM1 - 1))
                out_t = sb_out.tile([128, NCH], BF16, tag="out",
                                    name=f"out_{m}_{j}")
                qc = q[m][:, j * NCH:(j + 1) * NCH]
                if j == 0:
                    nc.vector.scalar_tensor_tensor(
                        out_t[:], pmm[:], a3c[:, m:m + 1], qc,
                        OP.mult, OP.add)
                else:
                    # ACT scales out of psum, DVE adds the shortcut (2x)
                    nc.scalar.activation(out_t[:], pmm[:], ACT.Copy,
                                         scale=a3c[:, m:m + 1])
                    nc.vector.tensor_tensor(out_t[:], out_t[:], qc, OP.add)
                nc.sync.dma_start(
                    out_d.ap()[m * 128:(m + 1) * 128,
                               j * NCH:(j + 1) * NCH],
                    out_t[:])


def _prep_host(inputs):
    """Host-side: shard x, fold weights, build per-core in_maps."""
    x = np.asarray(inputs["x"], np.float32)          # [64, 512, 28, 28]
    w1 = np.asarray(inputs["w1"], np.float32)
    w2 = np.asarray(inputs["w2"], np.float32)
    w3 = np.asarray(inputs["w3"], np.float32)
    ds_w = np.asarray(inputs["ds_w"], np.float32)

    s1, al1 = _ternarize_host(w1)    # [256,512,1,1]
    s2, al2 = _ternarize_host(w2)    # [256,256,3,3]
    s3, al3 = _ternarize_host(w3)    # [1024,256,1,1]

    t1 = np.ascontiguousarray(s1[:, :, 0, 0].T).astype(BF)       # [512, 256]
    # t2[t, i, o] = s2[o, i, ky, kx], t = ky*3+kx; shipped pre-tiled as
    # [k, i_local, t*C1 + o] so each partition row is one contiguous DMA
    t2 = s2.transpose(2, 3, 1, 0).reshape(9, C1, C1)
    s2sum = t2.sum(axis=0).astype(BF)                            # [256, 256]
    t2k = np.ascontiguousarray(
        t2.transpose(1, 0, 2).reshape(KM1, 128, 9 * C1)).astype(BF)
    t3 = np.ascontiguousarray(s3[:, :, 0, 0].T).astype(BF)       # [256, 1024]
    dsw = np.ascontiguousarray(ds_w[:, :, 0, 0].T).astype(BF)    # [512, 1024]

    def cols(v, n):
        # [n*128] channel vector -> [128, n] column layout
        return np.asarray(v, np.float32).reshape(n, 128).T

    par = np.concatenate([
        cols(al1, 2), cols(al2, 2), cols(al3, 8),
        cols(inputs["bn1_g"], 4), cols(inputs["bn2_g"], 2),
        cols(inputs["bn2_b"], 2), cols(inputs["bn3_g"], 2),
        cols(inputs["bn3_b"], 2), cols(inputs["ds_bn_g"], 8),
        cols(inputs["ds_bn_b"], 8),
    ], axis=1)
    par = np.ascontiguousarray(par, dtype=np.float32)            # [128, 40]

    common = dict(t1=t1, t2=t2k, s2sum=s2sum, t3=t3, dsw=dsw, par=par)

    in_maps = []
    for c in range(NCORES):
        xc = x[c * NI:(c + 1) * NI]                      # [8, 512, 28, 28]
        xc = np.ascontiguousarray(
            xc.transpose(1, 0, 2, 3).reshape(C0, PIX1)).astype(BF)
        in_maps.append({"x": xc, **common})
    return in_maps


def kernel(**inputs):
    if "nc" not in _CACHE:
        _CACHE["nc"] = build_program()
    nc = _CACHE["nc"]

    in_maps = _prep_host(inputs)
    try:
        res = run_bass_kernel_spmd(nc, in_maps, core_ids=list(range(NCORES)))
    except Exception:
        # transient device state (e.g. a previous crashed run) usually
        # clears on retry
        res = run_bass_kernel_spmd(nc, in_maps, core_ids=list(range(NCORES)))

    out = np.empty((64, C2, H2, H2), np.float32)
    for c in range(NCORES):
        oc = np.asarray(res.results[c]["out"]).astype(np.float32)
        oc = oc.reshape(C2, NI, H2, H2)
        out[c * NI:(c + 1) * NI] = oc.transpose(1, 0, 2, 3)
    return out
